# revision 1
# baseline (speedup 1.0000x reference)
"""DMV inside algorithm (Eisner chart DP, logsumexp semiring) on Trainium2.

Strategy
--------
Pure data parallelism over the batch: 4096 sentences -> 8 cores x 512.
Per core: 2 sequential "runs" of 256 sentences, each laid out as
[128 SBUF partitions] x [G=2 sentence groups in the free dim].

The DP runs in the *exp domain* (no per-split transcendentals): tables hold
exp(score) and each width-w update is one fused strided multiply + one fused
segmented reduce on VectorE, plus a handful of small fixup ops.

Tables are stored *diagonal-packed*: Xd[d*41 + i] = X[i, i+d], which makes
every gather in the width-w recurrence a regular (constant-stride) access
pattern. IR/IL are stored with row r holding width r+1 (IL additionally
column-shifted by +1) so that all four quantities' gathers share one AP.

Numerics: scale composes linearly in span width (every width-w entry contains
exactly w arcs), so on-device renormalization at w=14 and w=28 multiplies
row d by exp(delta*d) (and the per-arc constant tables by exp(delta)),
keeping everything in f32 range. The accumulated delta is returned per
sentence and undone on the host: LL = log(CR[0,len]) - dsum*len.
"""

import os

os.environ.setdefault("JAX_PLATFORMS", "cpu")

import numpy as np

import concourse.bass as bass  # noqa: F401  (registers engine classes)
import concourse.tile as tile
import bass_rust
from concourse import bacc, mybir

F32 = mybir.dt.float32
AF = mybir.ActivationFunctionType
OP = mybir.AluOpType
AX = mybir.AxisListType

N = 41              # fake_len (ROOT at 0)
D = 1681            # table pitch: N*N elements
G = 2               # sentence groups per partition
RUNS = 2            # runs per core (2 * 128 * G = 512 sentences)
NCORES = 8
B_CORE = RUNS * 128 * G
CONST_IN = 4 * D          # host sends 4 pre-exponentiated tables/sentence
STOP_IN = 8 * N           # host sends 8 exp'd stop/go vectors/sentence
RENORM_AT = (20,)

# banks tile: 8 diag-packed tables x 2 groups; slots arranged in 4 blocks of
# 4 so each big op's (q,g) gather is ONE fused AP dim (ISA: max 3 free dims):
#   opA in0: [KR_g0 KR_g1 CRa_g0 CRa_g1]  slots 0..3
#   opA in1: [CLb_g0 CLb_g1 KL_g0 KL_g1]  slots 4..7
#   opB in0: [IR_g0 IR_g1 CLa_g0 CLa_g1]  slots 8..11
#   opB in1: [CRb_g0 CRb_g1 IL_g0 IL_g1]  slots 12..15 (IL stored col+1)
S_KR, S_CRA, S_CLB, S_KL, S_IR, S_CLA, S_CRB, S_IL = (
    0, 2, 4, 6, 8, 10, 12, 14)
# consts tile: 4 per-arc tables, g-major: offset (4*g + C)*D
C_A1, C_B1, C_DA, C_DB = range(4)
# stops tile: 16 vectors of 41, offset (g*8 + v)*41
V_GL0, V_GL1, V_GR0, V_GR1, V_SLNO, V_SLHAS, V_SRNO, V_SRHAS = range(8)

# scratch tile element offsets
Z_P = 0          # 1680: products [qg,i,t]
Z_SSUM = 1680    # 164
Z_TMP1 = 1844    # 164
Z_TMP2 = 2008    # 164
Z_M2 = 2172      # 4
Z_MU = 2176      # 2
Z_LM = 2178      # 2 (reused for delta)
Z_M = 2180       # 84: renorm multiplier table [g, 42]
Z_CROUT = 2264   # 82
Z_DSUM = 2346    # 2
Z_IOTA = 2348    # 42
Z_TOTAL = 2390

LN2_32 = 32.0 * float(np.log(2.0))


def ap_of(t, offset, dims, lead=None):
    """Build a raw AP on tile/dram ap `t`: [lead or t.ap[0]] + dims."""
    ap = t.copy()
    first = list(t.ap[0]) if lead is None else list(lead)
    ap.ap = bass_rust.VecI64Pair([first] + [list(d) for d in dims])
    ap.offset = offset
    return ap


def build_nc():
    nc = bacc.Bacc("TRN2", target_bir_lowering=False, debug=False, num_devices=1)
    consts_in = nc.dram_tensor("consts", [B_CORE, CONST_IN], F32, kind="ExternalInput").ap()
    stops_in = nc.dram_tensor("stops", [B_CORE, STOP_IN], F32, kind="ExternalInput").ap()
    iota_d = nc.dram_tensor("iota", [42], F32, kind="ExternalInput").ap()
    logs_d = nc.dram_tensor("ecr", [B_CORE, N], F32, kind="ExternalOutput").ap()
    dsum_d = nc.dram_tensor("dsum", [B_CORE], F32, kind="ExternalOutput").ap()

    with tile.TileContext(nc) as tc:
        with tc.tile_pool(name="p", bufs=1) as pool:
            banks_t = pool.tile([128, 16 * D], F32)
            consts_t = pool.tile([128, 8 * D], F32)
            stops_t = pool.tile([128, 16 * N], F32)
            z_t = pool.tile([128, Z_TOTAL], F32)
            banks = banks_t[:]
            consts = consts_t[:]
            stops = stops_t[:]
            z = z_t[:]

            v = nc.vector
            sc = nc.scalar

            # iota constant (once)
            nc.sync.dma_start(
                ap_of(z, Z_IOTA, [[1, 42]]),
                ap_of(iota_d, 0, [[1, 42]], lead=[0, 128]),
            )

            for r in range(RUNS):
                base_s = r * 256  # first sentence of this run (per core)

                # ---- load host-precomputed exp-domain constants ----
                nc.sync.dma_start(
                    ap_of(stops, 0, [[STOP_IN, G], [1, STOP_IN]]),
                    ap_of(stops_in, base_s * STOP_IN,
                          [[STOP_IN, G], [1, STOP_IN]], lead=[G * STOP_IN, 128]),
                )
                # consts split by row range: step w reads row w only, so
                # later chunks' DMA hides under early DP steps
                for lo, hi in ((0, 2 * N), (2 * N, 8 * N), (8 * N, D)):
                    for g in range(G):
                        nc.sync.dma_start(
                            ap_of(consts, 4 * g * D + lo, [[D, 4], [1, hi - lo]]),
                            ap_of(consts_in, (base_s + g) * CONST_IN + lo,
                                  [[D, 4], [1, hi - lo]], lead=[G * CONST_IN, 128]),
                        )

                # ---- width-0 init ----
                # zero only rows the in-place renorm rescale can read before
                # the DP writes them (rows <= RENORM_AT[-1]+1); all gather
                # reads stay within written data by construction.
                nc.gpsimd.memset(
                    ap_of(banks, 0, [[D, 16], [1, (RENORM_AT[-1] + 2) * N]]), 0.0)
                v.memset(ap_of(z, Z_DSUM, [[1, 2]]), 0.0)
                # KR[0,:] = KL[0,:] = 1
                v.memset(ap_of(banks, S_KR * D, [[6 * D, 2], [D, 2], [1, N]]), 1.0)
                # CRa/CRb[0,i] = exp(stop[i,R,NO])
                v.tensor_copy(
                    ap_of(banks, S_CRA * D, [[10 * D, 2], [D, 2], [1, N]]),
                    ap_of(stops, V_SRNO * N, [[0, 2], [8 * N, 2], [1, N]]),
                )
                # CLa/CLb[0,i] = exp(stop[i,L,NO])
                v.tensor_copy(
                    ap_of(banks, S_CLA * D, [[-6 * D, 2], [D, 2], [1, N]]),
                    ap_of(stops, V_SLNO * N, [[0, 2], [8 * N, 2], [1, N]]),
                )

                # ---- chart DP ----
                for w in range(1, N):
                    s = N - w
                    row = (w - 1) * N + 1
                    # opA: P[qg,i,t] = {KR,CRa}[t,i] * {CLb,KL}[w-1-t, i+t+1]
                    pa = ap_of(z, Z_P, [[s * w, 4], [w, s], [1, w]])
                    v.tensor_tensor(
                        pa,
                        ap_of(banks, S_KR * D, [[D, 4], [1, s], [N, w]]),
                        ap_of(banks, S_CLB * D + row, [[D, 4], [1, s], [-40, w]]),
                        OP.mult,
                    )
                    v.reduce_sum(
                        ap_of(z, Z_SSUM, [[s, 4], [1, s]]), pa, axis=AX.X,
                    )
                    # tmp1 = Ssum * {A1,B1}[w,:]
                    v.tensor_tensor(
                        ap_of(z, Z_TMP1, [[2 * s, 2], [s, 2], [1, s]]),
                        ap_of(z, Z_SSUM, [[2 * s, 2], [s, 2], [1, s]]),
                        ap_of(consts, C_A1 * D + w * N, [[D, 2], [4 * D, 2], [1, s]]),
                        OP.mult,
                    )
                    # tmp2 = {CLb[w-1,1+i], CRa[w-1,i]} * {DA,DB}[w,:]
                    v.tensor_tensor(
                        ap_of(z, Z_TMP2, [[2 * s, 2], [s, 2], [1, s]]),
                        ap_of(banks, S_CLB * D + row, [[-2 * D - 1, 2], [D, 2], [1, s]]),
                        ap_of(consts, C_DA * D + w * N, [[D, 2], [4 * D, 2], [1, s]]),
                        OP.mult,
                    )
                    # IR[w-1, i] / IL[w-1, i+1] = tmp1 + tmp2
                    v.tensor_tensor(
                        ap_of(banks, S_IR * D + (w - 1) * N, [[6 * D + 1, 2], [D, 2], [1, s]]),
                        ap_of(z, Z_TMP1, [[2 * s, 2], [s, 2], [1, s]]),
                        ap_of(z, Z_TMP2, [[2 * s, 2], [s, 2], [1, s]]),
                        OP.add,
                    )
                    # opB: P[qg,i,t] = {IR,CLa}[t,i] * {CRb,IL}[w-1-t, i+t+1]
                    v.tensor_tensor(
                        pa,
                        ap_of(banks, S_IR * D, [[D, 4], [1, s], [N, w]]),
                        ap_of(banks, S_CRB * D + row, [[D, 4], [1, s], [-40, w]]),
                        OP.mult,
                    )
                    # KR[w,i], KL[w,i] = sum_t
                    v.reduce_sum(
                        ap_of(banks, S_KR * D + w * N, [[6 * D, 2], [D, 2], [1, s]]),
                        pa, axis=AX.X,
                    )
                    # CRa/CRb[w,i] = KR[w,i] * sRhas[i]
                    v.tensor_tensor(
                        ap_of(banks, S_CRA * D + w * N, [[10 * D, 2], [D, 2], [1, s]]),
                        ap_of(banks, S_KR * D + w * N, [[0, 2], [D, 2], [1, s]]),
                        ap_of(stops, V_SRHAS * N, [[0, 2], [8 * N, 2], [1, s]]),
                        OP.mult,
                    )
                    # CLa/CLb[w,i] = KL[w,i] * sLhas[i+w]
                    v.tensor_tensor(
                        ap_of(banks, S_CLA * D + w * N, [[-6 * D, 2], [D, 2], [1, s]]),
                        ap_of(banks, S_KL * D + w * N, [[0, 2], [D, 2], [1, s]]),
                        ap_of(stops, V_SLHAS * N + w, [[0, 2], [8 * N, 2], [1, s]]),
                        OP.mult,
                    )

                    if w in RENORM_AT:
                        s0 = N - w
                        # mu[g] = max_i max(KR[w,i], KL[w,i])
                        v.reduce_max(
                            ap_of(z, Z_M2, [[2, 2], [1, 2]]),
                            ap_of(banks, S_KR * D + w * N, [[6 * D, 2], [D, 2], [1, s0]]),
                            axis=AX.X,
                        )
                        v.tensor_tensor(
                            ap_of(z, Z_MU, [[1, 2]]),
                            ap_of(z, Z_M2, [[1, 2]]),
                            ap_of(z, Z_M2 + 2, [[1, 2]]),
                            OP.max,
                        )
                        # Ln range on ACT is +-2^64: compute via mu*2^-32
                        v.tensor_scalar_mul(
                            ap_of(z, Z_MU, [[1, 2]]), ap_of(z, Z_MU, [[1, 2]]), 2.0**-32
                        )
                        v.tensor_scalar_max(
                            ap_of(z, Z_MU, [[1, 2]]), ap_of(z, Z_MU, [[1, 2]]), 1e-36
                        )
                        sc.activation(
                            ap_of(z, Z_LM, [[1, 2]]), ap_of(z, Z_MU, [[1, 2]]), AF.Ln
                        )
                        # quantize the per-width shift to delta = -k*ln2 with
                        # k integer, so every rescale factor is an EXACT power
                        # of two (the ACT exp LUT would otherwise perturb all
                        # tables by its relative error).
                        # kf = round((log(mu*2^-32) + 32 ln2) / (w ln2))
                        v.tensor_scalar(
                            ap_of(z, Z_LM, [[1, 2]]), ap_of(z, Z_LM, [[1, 2]]),
                            LN2_32, 1.0 / (w * float(np.log(2.0))),
                            OP.add, OP.mult,
                        )
                        v.tensor_scalar(
                            ap_of(z, Z_LM, [[1, 2]]), ap_of(z, Z_LM, [[1, 2]]),
                            12582912.0, 12582912.0, OP.add, OP.subtract,
                        )
                        # dsum accumulates k (exact small integers)
                        v.tensor_tensor(
                            ap_of(z, Z_DSUM, [[1, 2]]),
                            ap_of(z, Z_DSUM, [[1, 2]]),
                            ap_of(z, Z_LM, [[1, 2]]),
                            OP.add,
                        )
                        # scale2 = 2^-k via exponent bits: (127 - k) << 23
                        v.tensor_scalar(
                            ap_of(z, Z_M2, [[1, 2]]), ap_of(z, Z_LM, [[1, 2]]),
                            -1.0, 127.0, OP.mult, OP.add,
                        )
                        zi = z.bitcast(mybir.dt.int32)
                        v.tensor_copy(
                            ap_of(zi, Z_M2 + 2, [[1, 2]]),
                            ap_of(z, Z_M2, [[1, 2]]),
                        )
                        v.tensor_scalar(
                            ap_of(zi, Z_M2 + 2, [[1, 2]]),
                            ap_of(zi, Z_M2 + 2, [[1, 2]]),
                            23, None, OP.arith_shift_left,
                        )
                        # M[g, d] = 2^(-k*d): d=0 -> 1, then multiplicative scan
                        v.memset(ap_of(z, Z_M, [[42, 2], [1, 1]]), 1.0)
                        for g in range(G):
                            sca = ap_of(z, Z_M2 + 2 + g, [[0, 41]])
                            v.tensor_tensor_scan(
                                ap_of(z, Z_M + g * 42 + 1, [[1, 41]]),
                                sca, sca, 1.0, OP.mult, OP.bypass,
                            )
                        for g in range(G):
                            # natural tables, rows d<=w: scale by exp(delta*d)
                            tA = ap_of(banks, g * D, [[2 * D, 4], [N, w + 1], [1, N]])
                            v.tensor_tensor(
                                tA, tA,
                                ap_of(z, Z_M + g * 42, [[0, 4], [1, w + 1], [0, N]]),
                                OP.mult,
                            )
                            tB = ap_of(banks, (10 + g) * D, [[2 * D, 2], [N, w + 1], [1, N]])
                            v.tensor_tensor(
                                tB, tB,
                                ap_of(z, Z_M + g * 42, [[0, 2], [1, w + 1], [0, N]]),
                                OP.mult,
                            )
                            # IR/IL rows r<=w-1 hold width r+1: exp(delta*(r+1))
                            tI = ap_of(banks, (8 + g) * D, [[6 * D, 2], [N, w], [1, N]])
                            v.tensor_tensor(
                                tI, tI,
                                ap_of(z, Z_M + g * 42 + 1, [[0, 2], [1, w], [0, N]]),
                                OP.mult,
                            )
                            # const rows > w: one extra arc factor exp(delta)
                            tC = ap_of(consts, 4 * g * D + (w + 1) * N,
                                       [[D, 4], [N, 40 - w], [1, N]])
                            v.tensor_tensor(
                                tC, tC,
                                ap_of(z, Z_M + g * 42 + 1, [[0, 4], [0, 40 - w], [0, N]]),
                                OP.mult,
                            )

                # ---- extract raw exp-domain CR[0, j] (log on host) ----
                v.tensor_copy(
                    ap_of(z, Z_CROUT, [[N, 2], [1, N]]),
                    ap_of(banks, S_CRA * D, [[D, 2], [N, N]]),
                )
                nc.sync.dma_start(
                    ap_of(logs_d, base_s * N, [[N, G], [1, N]], lead=[G * N, 128]),
                    ap_of(z, Z_CROUT, [[N, G], [1, N]]),
                )
                nc.sync.dma_start(
                    ap_of(dsum_d, base_s, [[1, G]], lead=[G, 128]),
                    ap_of(z, Z_DSUM, [[1, G]]),
                )

    nc.compile()
    return nc


_NC_CACHE = {}


def get_nc():
    if "nc" not in _NC_CACHE:
        _NC_CACHE["nc"] = build_nc()
    return _NC_CACHE["nc"]


def make_in_maps(trans_scores, dec_scores):
    t = np.asarray(trans_scores, dtype=np.float32)
    dec = np.asarray(dec_scores, dtype=np.float32)
    B = t.shape[0]
    go = dec[..., 0]                        # [B, n, dir, dv]
    # per-sentence linear pre-shift: each arc factor carries exp(-c0), so a
    # width-w entry is scaled exp(-c0*w); undone on the host at the end.
    tm = np.where(t < -1e8, -np.inf, t).max(axis=3)
    with np.errstate(invalid="ignore"):
        colmax = tm.max(axis=1)             # [B, n] best arc into each child
        proxy = np.nanmean(
            np.where(np.isfinite(colmax), colmax, np.nan)[:, 1:], axis=-1)
    c0 = (proxy + 0.5).astype(np.float32)
    c0 = np.clip(np.nan_to_num(c0), -20.0, 20.0)
    # one exp over trans (NEG -> 0 underflow is intended), then gather diags
    with np.errstate(under="ignore"):
        E = np.exp(t - c0[:, None, None, None])      # [B, n, n, 2]
        ego = np.exp(go)                             # [B, n, 2, 2]
    d_idx, i_idx = np.meshgrid(np.arange(N), np.arange(N), indexing="ij")
    j_idx = np.minimum(i_idx + d_idx, N - 1)
    valid = ((i_idx + d_idx) <= N - 1)[None].astype(np.float32)
    ea = E[:, i_idx, j_idx, :]              # [B, n, n, 2]  trans[i, i+d, v]
    eb = E[:, j_idx, i_idx, :]              # [B, n, n, 2]  trans[i+d, i, v]
    a1 = ea[..., 1] * ego[:, :, 1, 1][:, i_idx] * valid
    a0 = ea[..., 0] * ego[:, :, 1, 0][:, i_idx] * valid
    b1 = eb[..., 1] * ego[:, :, 0, 1][:, j_idx] * valid
    b0 = eb[..., 0] * ego[:, :, 0, 0][:, j_idx] * valid
    consts = np.empty((B, 4, N, N), dtype=np.float32)
    consts[:, 0] = a1
    consts[:, 1] = b1
    consts[:, 2] = a0 - a1
    consts[:, 3] = b0 - b1
    consts = consts.reshape(B, CONST_IN)
    est = np.exp(dec[..., 1])               # [B, n, dir, dv]
    stops = np.empty((B, 8, N), dtype=np.float32)
    stops[:, 0] = ego[:, :, 0, 0]; stops[:, 1] = ego[:, :, 0, 1]
    stops[:, 2] = ego[:, :, 1, 0]; stops[:, 3] = ego[:, :, 1, 1]
    stops[:, 4] = est[:, :, 0, 0]; stops[:, 5] = est[:, :, 0, 1]
    stops[:, 6] = est[:, :, 1, 0]; stops[:, 7] = est[:, :, 1, 1]
    stops = stops.reshape(B, STOP_IN)
    iota = np.arange(42, dtype=np.float32)
    in_maps = []
    for c in range(NCORES):
        sl = slice(c * B_CORE, (c + 1) * B_CORE)
        in_maps.append({
            "consts": consts[sl],
            "stops": stops[sl],
            "iota": iota,
        })
    return in_maps, c0


def assemble(results, len_array, c0):
    ln = np.asarray(len_array).astype(np.int64)
    c0 = np.asarray(c0).astype(np.float64)
    out = np.empty(len(ln), dtype=np.float32)
    for c, res in enumerate(results):
        ecr = res["ecr"].reshape(B_CORE, N).astype(np.float64)
        dsum = res["dsum"].reshape(B_CORE).astype(np.float64)
        lc = ln[c * B_CORE:(c + 1) * B_CORE]
        idx = np.arange(B_CORE)
        with np.errstate(divide="ignore"):
            out[c * B_CORE:(c + 1) * B_CORE] = (
                np.log(ecr[idx, lc]) + dsum * np.log(2.0) * lc
                + c0[c * B_CORE:(c + 1) * B_CORE] * lc
            ).astype(np.float32)
    return out


def kernel(trans_scores, dec_scores, len_array):
    from concourse.bass_utils import run_bass_kernel_spmd

    nc = get_nc()
    in_maps, c0 = make_in_maps(trans_scores, dec_scores)
    res = run_bass_kernel_spmd(nc, in_maps, core_ids=list(range(NCORES)))
    return assemble(res.results, len_array, c0)



# revision 12
# speedup vs baseline: 1.8231x; 1.8231x over previous
"""DMV inside algorithm (Eisner chart DP, logsumexp semiring) on Trainium2.

Strategy
--------
Pure data parallelism over the batch: 4096 sentences -> 8 cores x 512.
Per core: ONE run of 512 sentences laid out as [128 SBUF partitions] x
[G=4 sentence groups in the free dim], all chart tables in bf16.

The DP runs in the *exp domain* (no per-split transcendentals): tables hold
exp(score). Each width-w update is one fused strided multiply (products
P[qg,t,i]) followed by an in-place folding tree of adds that reduces over
the split dim t. Everything iterates [qg, t, i] with i innermost and
stride 1, which (with bf16) hits the DVE 2x packed-16-bit mode; the tree
of tensor_tensor adds also runs at 2x, unlike InstTensorReduce which gets
no fast mode.

Tables are stored *diagonal-packed*: Xd[d*41 + i] = X[i, i+d], making
every gather in the width-w recurrence a constant-stride access pattern.
IR/IL are stored with row r holding width r+1 (IL column-shifted by +1)
so all four quantities' gathers share one AP shape.

Numerics: scale composes linearly in span width (every width-w entry has
exactly w arcs), so one on-device renormalization at w=20 multiplies row
d by an exact power of two 2^(-k*d) (k integer per sentence), keeping
everything in range (bf16 range == f32 range). k is returned per sentence
and undone on the host: LL = log(CR[0,len]) + k*ln2*len + c0*len.
"""

import os

os.environ.setdefault("JAX_PLATFORMS", "cpu")

import numpy as np
import ml_dtypes

import concourse.bass as bass  # noqa: F401  (registers engine classes)
import concourse.tile as tile
import bass_rust
from concourse import bacc, mybir

F32 = mybir.dt.float32
BF16 = mybir.dt.bfloat16
AF = mybir.ActivationFunctionType
OP = mybir.AluOpType
AX = mybir.AxisListType

N = 41              # fake_len (ROOT at 0)
D = 1681            # table pitch: N*N elements
G = 4               # sentence groups per partition
NCORES = 8
B_CORE = 128 * G    # 512
CONST_IN = 4 * D    # host sends 4 pre-exponentiated tables/sentence (bf16)
STOP_IN = 8 * N     # host sends 8 exp'd stop/go vectors/sentence (bf16)
RENORM_W = 20

# banks tile (bf16): 32 diag-packed tables, slot k at offset k*D. ISA AP
# steps are 16-bit (<= 32767 = 19*D), so slots are arranged to keep every
# paired access within 16*D:
#   opA in1: slots  0..7  = [CLb g0..g3, KL  g0..g3]   (CLb stored col+1)
#   opA in0: slots  8..15 = [KR  g0..g3, CRa g0..g3]
#   opB in0: slots 16..23 = [CLa g0..g3, IR  g0..g3]
#   opB in1: slots 24..31 = [IL  g0..g3, CRb g0..g3]   (IL stored col+1)
# opA iterates q = [R, L]; opB iterates q = [L, R].
S_CLB, S_KL, S_KR, S_CRA, S_CLA, S_IR, S_IL, S_CRB = (
    0, 4, 8, 12, 16, 20, 24, 28)
# consts tile (bf16): 4 per-arc tables, g-major: offset (4*g + C)*D
# (A0/B0 kept verbatim, not as differences: all-positive arithmetic so
# bf16 never hits catastrophic cancellation)
C_A1, C_B1, C_A0, C_B0 = range(4)
# stops tile (bf16): 32 vectors of 41, offset (g*8 + v)*41
V_GL0, V_GL1, V_GR0, V_GR1, V_SLNO, V_SLHAS, V_SRNO, V_SRHAS = range(8)

# bf16 scratch tile element offsets
ZB_P = 0          # 3360: products [qg, t, i] (qg-stride = s*w, varies by w)
ZB_T1 = 3360      # 328: tmp1 [q, g, i]
ZB_T2 = 3688      # 328
ZB_MX = 4016      # 4*22*41: expanded renorm multiplier Mx[g, d, i] = 2^(-k_g*d)
ZB_TOTAL = 7624

# f32 scratch tile element offsets
ZF_M2 = 0         # 8
ZF_MU = 8         # 4
ZF_LM = 12        # 4 (reused for k)
ZF_M = 16         # 4*42: renorm multiplier table [g, 42]
ZF_CROUT = 184    # 4*41
ZF_DSUM = 348     # 4
ZF_TOTAL = 352

LN2_32 = 32.0 * float(np.log(2.0))


def ap_of(t, offset, dims, lead=None):
    """Build a raw AP on tile/dram ap `t`: [lead or t.ap[0]] + dims."""
    ap = t.copy()
    first = list(t.ap[0]) if lead is None else list(lead)
    ap.ap = bass_rust.VecI64Pair([first] + [list(d) for d in dims])
    ap.offset = offset
    return ap


def build_nc():
    nc = bacc.Bacc("TRN2", target_bir_lowering=False, debug=False, num_devices=1)
    consts_in = nc.dram_tensor("consts", [B_CORE, CONST_IN], BF16, kind="ExternalInput").ap()
    stops_in = nc.dram_tensor("stops", [B_CORE, STOP_IN], BF16, kind="ExternalInput").ap()
    logs_d = nc.dram_tensor("ecr", [B_CORE, N], F32, kind="ExternalOutput").ap()
    dsum_d = nc.dram_tensor("dsum", [B_CORE], F32, kind="ExternalOutput").ap()

    with tile.TileContext(nc) as tc:
        with tc.tile_pool(name="p", bufs=1) as pool:
            banks_t = pool.tile([128, 32 * D], BF16)
            consts_t = pool.tile([128, 16 * D], BF16)
            stops_t = pool.tile([128, 32 * N], BF16)
            zb_t = pool.tile([128, ZB_TOTAL], BF16)
            zf_t = pool.tile([128, ZF_TOTAL], F32)
            banks = banks_t[:]
            consts = consts_t[:]
            stops = stops_t[:]
            zb = zb_t[:]
            zf = zf_t[:]

            v = nc.vector
            sc = nc.scalar

            # ---- load host-precomputed exp-domain constants ----
            nc.sync.dma_start(
                ap_of(stops, 0, [[STOP_IN, G], [1, STOP_IN]]),
                ap_of(stops_in, 0,
                      [[STOP_IN, G], [1, STOP_IN]], lead=[G * STOP_IN, 128]),
            )
            # consts split by row range: step w reads row w only, so later
            # chunks' DMA hides under early DP steps
            for lo, hi in ((0, 2 * N), (2 * N, 5 * N), (5 * N, 10 * N),
                           (10 * N, 18 * N), (18 * N, 29 * N), (29 * N, D)):
                for g in range(G):
                    nc.sync.dma_start(
                        ap_of(consts, 4 * g * D + lo, [[D, 4], [1, hi - lo]]),
                        ap_of(consts_in, g * CONST_IN + lo,
                              [[D, 4], [1, hi - lo]], lead=[G * CONST_IN, 128]),
                    )

            # ---- width-0 init ----
            # The renorm rescale reads full N-wide rows whose tail columns the
            # DP never writes: zero exactly those tails (Pool, no dep with any
            # DP write, so nothing gates the chart loop).
            for d in range(1, RENORM_W + 1):
                # K/C-type tables: row d written on cols [0, N-d)
                nc.gpsimd.memset(
                    ap_of(banks, d * N + (N - d), [[D, 20], [1, d]]), 0.0)
                nc.gpsimd.memset(
                    ap_of(banks, 28 * D + d * N + (N - d), [[D, 4], [1, d]]), 0.0)
            for r in range(RENORM_W):
                # IR row r written on cols [0, N-r-1); IL on [1, N-r-1]
                nc.gpsimd.memset(
                    ap_of(banks, S_IR * D + r * N + (N - r - 1), [[D, 4], [1, r + 1]]), 0.0)
                if r >= 1:
                    nc.gpsimd.memset(
                        ap_of(banks, S_IL * D + r * N + (N - r), [[D, 4], [1, r]]), 0.0)
            nc.gpsimd.memset(
                ap_of(banks, S_IL * D, [[D, 4], [N, RENORM_W], [1, 1]]), 0.0)
            v.memset(ap_of(zf, ZF_DSUM, [[1, G]]), 0.0)
            # KR[0,:] = KL[0,:] = 1
            v.memset(ap_of(banks, S_KL * D, [[4 * D, 2], [D, 4], [1, N]]), 1.0)
            # CRa/CRb[0,i] = exp(stop[i,R,NO])
            v.tensor_copy(
                ap_of(banks, S_CRA * D, [[16 * D, 2], [D, 4], [1, N]]),
                ap_of(stops, V_SRNO * N, [[0, 2], [8 * N, 4], [1, N]]),
            )
            # CLa/CLb[0,i] = exp(stop[i,L,NO])
            v.tensor_copy(
                ap_of(banks, S_CLA * D, [[-16 * D, 2], [D, 4], [1, N]]),
                ap_of(stops, V_SLNO * N, [[0, 2], [8 * N, 4], [1, N]]),
            )

            def fold_pair(w, s, t0, count, final0=None, final1=None):
                """In-place fold the two 4-slot halves of P[qg, t, i] over t
                in [t0, t0+count) down to one row at t0, emitting the halves'
                levels interleaved so the independent chains hide each
                other's semaphore latency. final0/final1 redirect each
                half's last fold."""
                sw = s * w
                h = count
                while h > 1:
                    h2 = h // 2
                    hc = h - h2
                    for half, fin in ((0, final0), (1, final1)):
                        base = ZB_P + half * 4 * sw + t0 * s
                        if hc == 1 and fin is not None:
                            out = fin
                        else:
                            out = ap_of(zb, base, [[sw, 4], [s, h2], [1, s]])
                        v.tensor_tensor(
                            out,
                            ap_of(zb, base, [[sw, 4], [s, h2], [1, s]]),
                            ap_of(zb, base + hc * s, [[sw, 4], [s, h2], [1, s]]),
                            OP.add,
                        )
                    h = hc

            # ---- chart DP ----
            for w in range(1, N):
                s = N - w
                sw = s * w
                row = (w - 1) * N + 1
                # opA products, q=R forward: P[g,t,i] = KR[t,i]*CLb[w-1-t, i+t+1]
                # (t=0 term = CLb[w-1, i+1], the NOCHILD edge, since KR[0]=1)
                v.tensor_tensor(
                    ap_of(zb, ZB_P, [[sw, 4], [s, w], [1, s]]),
                    ap_of(banks, S_KR * D, [[D, 4], [N, w], [1, s]]),
                    ap_of(banks, S_CLB * D + row, [[D, 4], [-40, w], [1, s]]),
                    OP.mult,
                )
                # q=L t-reversed: P[4+g,t',i] = CRa[w-1-t',i]*KL[t', i+w-t']
                # (t'=0 term = CRa[w-1, i], the NOCHILD edge, since KL[0]=1)
                v.tensor_tensor(
                    ap_of(zb, ZB_P + 4 * sw, [[sw, 4], [s, w], [1, s]]),
                    ap_of(banks, S_CRA * D + (w - 1) * N, [[D, 4], [-N, w], [1, s]]),
                    ap_of(banks, S_KL * D + w, [[D, 4], [40, w], [1, s]]),
                    OP.mult,
                )
                if w > 1:
                    # tmp2[q,g,i] = NOCHILD edge * {A0,B0}[w,:] — depends only
                    # on the mults (P[:,0,:]), so it runs under the folds
                    v.tensor_tensor(
                        ap_of(zb, ZB_T2, [[4 * s, 2], [s, 4], [1, s]]),
                        ap_of(zb, ZB_P, [[4 * sw, 2], [sw, 4], [1, s]]),
                        ap_of(consts, C_A0 * D + w * N, [[D, 2], [4 * D, 4], [1, s]]),
                        OP.mult,
                    )
                    fold_pair(w, s, 1, w - 1)
                    # tmp1[q,g,i] = (sum over HASCHILD splits) * {A1,B1}[w,:]
                    v.tensor_tensor(
                        ap_of(zb, ZB_T1, [[4 * s, 2], [s, 4], [1, s]]),
                        ap_of(zb, ZB_P + s, [[4 * sw, 2], [sw, 4], [1, s]]),
                        ap_of(consts, C_A1 * D + w * N, [[D, 2], [4 * D, 4], [1, s]]),
                        OP.mult,
                    )
                    # IR[w-1, i] / IL[w-1, i+1] = tmp1 + tmp2
                    v.tensor_tensor(
                        ap_of(banks, S_IR * D + (w - 1) * N, [[4 * D + 1, 2], [D, 4], [1, s]]),
                        ap_of(zb, ZB_T1, [[4 * s, 2], [s, 4], [1, s]]),
                        ap_of(zb, ZB_T2, [[4 * s, 2], [s, 4], [1, s]]),
                        OP.add,
                    )
                else:
                    # w=1: only the NOCHILD edge exists
                    v.tensor_tensor(
                        ap_of(banks, S_IR * D + (w - 1) * N, [[4 * D + 1, 2], [D, 4], [1, s]]),
                        ap_of(zb, ZB_P, [[4 * sw, 2], [sw, 4], [1, s]]),
                        ap_of(consts, C_A0 * D + w * N, [[D, 2], [4 * D, 4], [1, s]]),
                        OP.mult,
                    )
                # opB products, half 0 (q=L): P[g,t,i] = CLa[t,i]*IL[w-1-t, i+t+1]
                # half 1 (q=R): P[4+g,t,i] = IR[t,i]*CRb[w-1-t, i+t+1]
                klout = ap_of(banks, S_KL * D + w * N, [[D, 4], [1, s]])
                krout = ap_of(banks, S_KR * D + w * N, [[D, 4], [1, s]])
                if w == 1:
                    v.tensor_tensor(
                        klout,
                        ap_of(banks, S_CLA * D, [[D, 4], [N, 1], [1, s]]),
                        ap_of(banks, S_IL * D + row, [[D, 4], [-40, 1], [1, s]]),
                        OP.mult,
                    )
                    v.tensor_tensor(
                        krout,
                        ap_of(banks, S_IR * D, [[D, 4], [N, 1], [1, s]]),
                        ap_of(banks, S_CRB * D + row, [[D, 4], [-40, 1], [1, s]]),
                        OP.mult,
                    )
                else:
                    v.tensor_tensor(
                        ap_of(zb, ZB_P, [[sw, 4], [s, w], [1, s]]),
                        ap_of(banks, S_CLA * D, [[D, 4], [N, w], [1, s]]),
                        ap_of(banks, S_IL * D + row, [[D, 4], [-40, w], [1, s]]),
                        OP.mult,
                    )
                    v.tensor_tensor(
                        ap_of(zb, ZB_P + 4 * sw, [[sw, 4], [s, w], [1, s]]),
                        ap_of(banks, S_IR * D, [[D, 4], [N, w], [1, s]]),
                        ap_of(banks, S_CRB * D + row, [[D, 4], [-40, w], [1, s]]),
                        OP.mult,
                    )
                    fold_pair(w, s, 0, w, final0=klout, final1=krout)
                # CRa/CRb[w,i] = KR[w,i] * sRhas[i]
                v.tensor_tensor(
                    ap_of(banks, S_CRA * D + w * N, [[16 * D, 2], [D, 4], [1, s]]),
                    ap_of(banks, S_KR * D + w * N, [[0, 2], [D, 4], [1, s]]),
                    ap_of(stops, V_SRHAS * N, [[0, 2], [8 * N, 4], [1, s]]),
                    OP.mult,
                )
                # CLa/CLb[w,i] = KL[w,i] * sLhas[i+w]
                v.tensor_tensor(
                    ap_of(banks, S_CLA * D + w * N, [[-16 * D, 2], [D, 4], [1, s]]),
                    ap_of(banks, S_KL * D + w * N, [[0, 2], [D, 4], [1, s]]),
                    ap_of(stops, V_SLHAS * N + w, [[0, 2], [8 * N, 4], [1, s]]),
                    OP.mult,
                )

                if w == RENORM_W:
                    s0 = N - w
                    # mu[g] = max_i max(KR[w,i], KL[w,i])  (per partition)
                    v.tensor_reduce(
                        ap_of(zf, ZF_M2, [[4, 2], [1, 4]]),
                        ap_of(banks, S_KL * D + w * N, [[4 * D, 2], [D, 4], [1, s0]]),
                        axis=AX.X, op=OP.max,
                    )
                    v.tensor_tensor(
                        ap_of(zf, ZF_MU, [[1, 4]]),
                        ap_of(zf, ZF_M2, [[1, 4]]),
                        ap_of(zf, ZF_M2 + 4, [[1, 4]]),
                        OP.max,
                    )
                    # Ln range on ACT is +-2^64: compute via mu*2^-32
                    v.tensor_scalar_mul(
                        ap_of(zf, ZF_MU, [[1, 4]]), ap_of(zf, ZF_MU, [[1, 4]]), 2.0**-32
                    )
                    v.tensor_scalar_max(
                        ap_of(zf, ZF_MU, [[1, 4]]), ap_of(zf, ZF_MU, [[1, 4]]), 1e-36
                    )
                    sc.activation(
                        ap_of(zf, ZF_LM, [[1, 4]]), ap_of(zf, ZF_MU, [[1, 4]]), AF.Ln
                    )
                    # quantize the per-width shift to delta = -k*ln2 with k
                    # integer, so every rescale factor is an EXACT power of
                    # two (exact in bf16 as well).
                    # kf = round((log(mu*2^-32) + 32 ln2) / (w ln2))
                    v.tensor_scalar(
                        ap_of(zf, ZF_LM, [[1, 4]]), ap_of(zf, ZF_LM, [[1, 4]]),
                        LN2_32, 1.0 / (w * float(np.log(2.0))),
                        OP.add, OP.mult,
                    )
                    v.tensor_scalar(
                        ap_of(zf, ZF_LM, [[1, 4]]), ap_of(zf, ZF_LM, [[1, 4]]),
                        12582912.0, 12582912.0, OP.add, OP.subtract,
                    )
                    # dsum accumulates k (exact small integers)
                    v.tensor_tensor(
                        ap_of(zf, ZF_DSUM, [[1, 4]]),
                        ap_of(zf, ZF_DSUM, [[1, 4]]),
                        ap_of(zf, ZF_LM, [[1, 4]]),
                        OP.add,
                    )
                    # scale2 = 2^-k via exponent bits: (127 - k) << 23
                    v.tensor_scalar(
                        ap_of(zf, ZF_M2, [[1, 4]]), ap_of(zf, ZF_LM, [[1, 4]]),
                        -1.0, 127.0, OP.mult, OP.add,
                    )
                    zi = zf.bitcast(mybir.dt.int32)
                    v.tensor_copy(
                        ap_of(zi, ZF_M2 + 4, [[1, 4]]),
                        ap_of(zf, ZF_M2, [[1, 4]]),
                    )
                    v.tensor_scalar(
                        ap_of(zi, ZF_M2 + 4, [[1, 4]]),
                        ap_of(zi, ZF_M2 + 4, [[1, 4]]),
                        23, None, OP.arith_shift_left,
                    )
                    # M[g, d] = 2^(-k*d): d=0 -> 1, then multiplicative scan
                    v.memset(ap_of(zf, ZF_M, [[42, 4], [1, 1]]), 1.0)
                    for g in range(G):
                        sca = ap_of(zf, ZF_M2 + 4 + g, [[0, 41]])
                        v.tensor_tensor_scan(
                            ap_of(zf, ZF_M + g * 42 + 1, [[1, 41]]),
                            sca, sca, 1.0, OP.mult, OP.bypass,
                        )
                    # expand to Mx[g, d, i] = M[g, d] (packed bf16, exact
                    # powers of two) so the rescales hit the 2x DVE mode
                    v.tensor_copy(
                        ap_of(zb, ZB_MX, [[(w + 2) * N, 4], [N, w + 2], [1, N]]),
                        ap_of(zf, ZF_M, [[42, 4], [1, w + 2], [0, N]]),
                    )
                    # far const rows (needed from w=26 on) rescale on Pool,
                    # overlapping the DVE rescales and the next few widths
                    for g in range(G):
                        tCf = ap_of(consts, 4 * g * D + 26 * N,
                                    [[D, 4], [N, N - 26], [1, N]])
                        nc.gpsimd.tensor_tensor(
                            tCf, tCf,
                            ap_of(zb, ZB_MX + g * (w + 2) * N + N,
                                  [[0, 4], [0, N - 26], [1, N]]),
                            OP.mult,
                        )
                    for g in range(G):
                        mg = ZB_MX + g * (w + 2) * N
                        # natural tables, rows d<=w: scale by 2^(-k*d)
                        tA = ap_of(banks, g * D, [[4 * D, 4], [N, w + 1], [1, N]])
                        v.tensor_tensor(
                            tA, tA,
                            ap_of(zb, mg, [[0, 4], [N, w + 1], [1, N]]),
                            OP.mult,
                        )
                        tB = ap_of(banks, (16 + g) * D, [[12 * D, 2], [N, w + 1], [1, N]])
                        v.tensor_tensor(
                            tB, tB,
                            ap_of(zb, mg, [[0, 2], [N, w + 1], [1, N]]),
                            OP.mult,
                        )
                        # IR/IL rows r<=w-1 hold width r+1: scale 2^(-k*(r+1))
                        tI = ap_of(banks, (20 + g) * D, [[4 * D, 2], [N, w], [1, N]])
                        v.tensor_tensor(
                            tI, tI,
                            ap_of(zb, mg + N, [[0, 2], [N, w], [1, N]]),
                            OP.mult,
                        )
                        # near const rows w+1..25: one extra arc factor 2^-k
                        tC = ap_of(consts, 4 * g * D + (w + 1) * N,
                                   [[D, 4], [N, 25 - w], [1, N]])
                        v.tensor_tensor(
                            tC, tC,
                            ap_of(zb, mg + N, [[0, 4], [0, 25 - w], [1, N]]),
                            OP.mult,
                        )

            # ---- extract raw exp-domain CR[0, j] (log on host) ----
            v.tensor_copy(
                ap_of(zf, ZF_CROUT, [[N, 4], [1, N]]),
                ap_of(banks, S_CRA * D, [[D, 4], [N, N]]),
            )
            nc.sync.dma_start(
                ap_of(logs_d, 0, [[N, G], [1, N]], lead=[G * N, 128]),
                ap_of(zf, ZF_CROUT, [[N, G], [1, N]]),
            )
            nc.sync.dma_start(
                ap_of(dsum_d, 0, [[1, G]], lead=[G, 128]),
                ap_of(zf, ZF_DSUM, [[1, G]]),
            )

    nc.compile()
    return nc


_NC_CACHE = {}


def get_nc():
    if "nc" not in _NC_CACHE:
        _NC_CACHE["nc"] = build_nc()
    return _NC_CACHE["nc"]


def make_in_maps(trans_scores, dec_scores):
    t = np.asarray(trans_scores, dtype=np.float32)
    dec = np.asarray(dec_scores, dtype=np.float32)
    B = t.shape[0]
    go = dec[..., 0]                        # [B, n, dir, dv]
    # per-sentence linear pre-shift: each arc factor carries exp(-c0), so a
    # width-w entry is scaled exp(-c0*w); undone on the host at the end.
    tm = np.where(t < -1e8, -np.inf, t).max(axis=3)
    with np.errstate(invalid="ignore"):
        colmax = tm.max(axis=1)             # [B, n] best arc into each child
        proxy = np.nanmean(
            np.where(np.isfinite(colmax), colmax, np.nan)[:, 1:], axis=-1)
    c0 = (proxy + 0.5).astype(np.float32)
    c0 = np.clip(np.nan_to_num(c0), -20.0, 20.0)
    # one exp over trans (NEG -> 0 underflow is intended), then gather diags
    with np.errstate(under="ignore"):
        E = np.exp(t - c0[:, None, None, None])      # [B, n, n, 2]
        ego = np.exp(go)                             # [B, n, 2, 2]
    d_idx, i_idx = np.meshgrid(np.arange(N), np.arange(N), indexing="ij")
    j_idx = np.minimum(i_idx + d_idx, N - 1)
    valid = ((i_idx + d_idx) <= N - 1)[None].astype(np.float32)
    ea = E[:, i_idx, j_idx, :]              # [B, n, n, 2]  trans[i, i+d, v]
    eb = E[:, j_idx, i_idx, :]              # [B, n, n, 2]  trans[i+d, i, v]
    a1 = ea[..., 1] * ego[:, :, 1, 1][:, i_idx] * valid
    a0 = ea[..., 0] * ego[:, :, 1, 0][:, i_idx] * valid
    b1 = eb[..., 1] * ego[:, :, 0, 1][:, j_idx] * valid
    b0 = eb[..., 0] * ego[:, :, 0, 0][:, j_idx] * valid
    consts = np.empty((B, 4, N, N), dtype=np.float32)
    consts[:, 0] = a1
    consts[:, 1] = b1
    consts[:, 2] = a0
    consts[:, 3] = b0
    consts = consts.reshape(B, CONST_IN).astype(ml_dtypes.bfloat16)
    est = np.exp(dec[..., 1])               # [B, n, dir, dv]
    stops = np.empty((B, 8, N), dtype=np.float32)
    stops[:, 0] = ego[:, :, 0, 0]; stops[:, 1] = ego[:, :, 0, 1]
    stops[:, 2] = ego[:, :, 1, 0]; stops[:, 3] = ego[:, :, 1, 1]
    stops[:, 4] = est[:, :, 0, 0]; stops[:, 5] = est[:, :, 0, 1]
    stops[:, 6] = est[:, :, 1, 0]; stops[:, 7] = est[:, :, 1, 1]
    stops = stops.reshape(B, STOP_IN).astype(ml_dtypes.bfloat16)
    in_maps = []
    for c in range(NCORES):
        sl = slice(c * B_CORE, (c + 1) * B_CORE)
        in_maps.append({
            "consts": consts[sl],
            "stops": stops[sl],
        })
    return in_maps, c0


L0_HOST = 5  # sentences with len <= L0_HOST are computed exactly on the host


def _host_short_ll(trans, dec, lens):
    """Exact f64 LL for short sentences via the inside DP truncated to
    positions 0..L0_HOST (spans of a length-l sentence live within [0, l])."""
    n = L0_HOST + 1
    t = np.asarray(trans)[:, :n, :n, :].astype(np.float64)
    dc = np.asarray(dec)[:, :n].astype(np.float64)
    B = t.shape[0]
    go = dc[..., 0]
    stop = dc[..., 1]
    NEG = -1e9
    IR = np.full((B, n, n), NEG)
    IL = np.full((B, n, n), NEG)
    KR = np.full((B, n, n), NEG)
    KL = np.full((B, n, n), NEG)
    dg = np.arange(n)
    KR[:, dg, dg] = 0.0
    KL[:, dg, dg] = 0.0
    CR = np.full((B, n, n), NEG)
    CL = np.full((B, n, n), NEG)
    CR[:, dg, dg] = stop[:, :, 1, 0]
    CL[:, dg, dg] = stop[:, :, 0, 0]
    goR = go[:, :, 1, :]
    goL = go[:, :, 0, :]

    def lse(x):
        m = x.max(axis=-1, keepdims=True)
        return np.squeeze(m, -1) + np.log(np.exp(x - m).sum(axis=-1))

    for w in range(1, n):
        s = n - w
        i = np.arange(s)[:, None]
        tt = np.arange(w)[None, :]
        j = i + w
        k = i + tt
        vR = (tt > 0).astype(np.int64)
        ir = lse(KR[:, i, k] + goR[:, i, vR] + t[:, i, j, np.minimum(vR, 1)]
                 + CL[:, k + 1, j])
        vL = (tt < w - 1).astype(np.int64)
        il = lse(CR[:, i, k] + KL[:, k + 1, j] + goL[:, j, vL]
                 + t[:, j, i, np.minimum(vL, 1)])
        i1 = np.arange(s)
        IR[:, i1, i1 + w] = ir
        IL[:, i1, i1 + w] = il
        kr = lse(IR[:, i, i + 1 + tt] + CR[:, i + 1 + tt, j])
        kl = lse(CL[:, i, i + tt] + IL[:, i + tt, j])
        KR[:, i1, i1 + w] = kr
        KL[:, i1, i1 + w] = kl
        CR[:, i1, i1 + w] = kr + stop[:, i1, 1, 1]
        CL[:, i1, i1 + w] = kl + stop[:, i1 + w, 0, 1]

    return CR[np.arange(B), 0, lens].astype(np.float32)


def assemble(results, len_array, c0):
    ln = np.asarray(len_array).astype(np.int64)
    c0 = np.asarray(c0).astype(np.float64)
    out = np.empty(len(ln), dtype=np.float32)
    for c, res in enumerate(results):
        ecr = res["ecr"].reshape(B_CORE, N).astype(np.float64)
        dsum = res["dsum"].reshape(B_CORE).astype(np.float64)
        lc = ln[c * B_CORE:(c + 1) * B_CORE]
        idx = np.arange(B_CORE)
        with np.errstate(divide="ignore"):
            out[c * B_CORE:(c + 1) * B_CORE] = (
                np.log(ecr[idx, lc]) + dsum * np.log(2.0) * lc
                + c0[c * B_CORE:(c + 1) * B_CORE] * lc
            ).astype(np.float32)
    return out


def kernel(trans_scores, dec_scores, len_array):
    from concourse.bass_utils import run_bass_kernel_spmd

    nc = get_nc()
    in_maps, c0 = make_in_maps(trans_scores, dec_scores)
    res = run_bass_kernel_spmd(nc, in_maps, core_ids=list(range(NCORES)))
    out = assemble(res.results, len_array, c0)
    lens = np.asarray(len_array).astype(np.int64)
    short = lens <= L0_HOST
    if short.any():
        out[short] = _host_short_ll(
            np.asarray(trans_scores)[short], np.asarray(dec_scores)[short],
            lens[short])
    return out


# revision 13
# speedup vs baseline: 1.8613x; 1.0209x over previous
"""DMV inside algorithm (Eisner chart DP, logsumexp semiring) on Trainium2.

Strategy
--------
Pure data parallelism over the batch: 4096 sentences -> 8 cores x 512.
Per core: ONE run of 512 sentences laid out as [128 SBUF partitions] x
[G=4 sentence groups in the free dim], all chart tables in bf16.

The DP runs in the *exp domain* (no per-split transcendentals): tables hold
exp(score). Each width-w update is one fused strided multiply (products
P[qg,t,i]) followed by an in-place folding tree of adds that reduces over
the split dim t. Everything iterates [qg, t, i] with i innermost and
stride 1, which (with bf16) hits the DVE 2x packed-16-bit mode; the tree
of tensor_tensor adds also runs at 2x, unlike InstTensorReduce which gets
no fast mode.

Tables are stored *diagonal-packed*: Xd[d*41 + i] = X[i, i+d], making
every gather in the width-w recurrence a constant-stride access pattern.
IR/IL are stored with row r holding width r+1 (IL column-shifted by +1)
so all four quantities' gathers share one AP shape.

Numerics: scale composes linearly in span width (every width-w entry has
exactly w arcs), so one on-device renormalization at w=20 multiplies row
d by an exact power of two 2^(-k*d) (k integer per sentence), keeping
everything in range (bf16 range == f32 range). k is returned per sentence
and undone on the host: LL = log(CR[0,len]) + k*ln2*len + c0*len.
"""

import os

os.environ.setdefault("JAX_PLATFORMS", "cpu")

import numpy as np
import ml_dtypes

import concourse.bass as bass  # noqa: F401  (registers engine classes)
import concourse.tile as tile
import bass_rust
from concourse import bacc, mybir

F32 = mybir.dt.float32
BF16 = mybir.dt.bfloat16
AF = mybir.ActivationFunctionType
OP = mybir.AluOpType
AX = mybir.AxisListType

N = 41              # fake_len (ROOT at 0)
D = 1681            # table pitch: N*N elements
G = 4               # sentence groups per partition
NCORES = 8
B_CORE = 128 * G    # 512
CONST_IN = 4 * D    # host sends 4 pre-exponentiated tables/sentence (bf16)
STOP_IN = 8 * N     # host sends 8 exp'd stop/go vectors/sentence (bf16)
RENORM_W = 20

# banks tile (bf16): 24 diag-packed tables, slot k at offset k*D.
# Since every DP multiply is a per-direction 4-slot op, no table needs a
# second copy; all paired-access strides stay within 16*D (ISA AP steps
# are 16-bit).
#   KL g0..g3: 0..3    KR: 4..7    CRa: 8..11   CLa: 12..15
#   IR: 16..19         IL: 20..23  (IL stored col+1)
S_KL, S_KR, S_CRA, S_CLA, S_IR, S_IL = 0, 4, 8, 12, 16, 20
# consts tile (bf16): 4 per-arc tables, g-major: offset (4*g + C)*D
# (A0/B0 kept verbatim, not as differences: all-positive arithmetic so
# bf16 never hits catastrophic cancellation)
C_A1, C_B1, C_A0, C_B0 = range(4)
# stops tile (bf16): 32 vectors of 41, offset (g*8 + v)*41
V_GL0, V_GL1, V_GR0, V_GR1, V_SLNO, V_SLHAS, V_SRNO, V_SRHAS = range(8)

# bf16 scratch tile element offsets
ZB_P = 0          # 3360: products [qg, t, i] (qg-stride = s*w, varies by w)
ZB_T1 = 3360      # 328: tmp1 [q, g, i]
ZB_T2 = 3688      # 328
ZB_SS = 4016      # 328: reduce-path sums [q, g, i]
ZB_MX = 4344      # 4*22*41: expanded renorm multiplier Mx[g, d, i] = 2^(-k_g*d)
ZB_TOTAL = 7952

# f32 scratch tile element offsets
ZF_M2 = 0         # 8
ZF_MU = 8         # 4
ZF_LM = 12        # 4 (reused for k)
ZF_M = 16         # 4*42: renorm multiplier table [g, 42]
ZF_CROUT = 184    # 4*41
ZF_DSUM = 348     # 4
ZF_TOTAL = 352

LN2_32 = 32.0 * float(np.log(2.0))

# below this many (elements per fold-chain * levels) a single TensorReduce
# (1x rate but one instruction) beats the 2x-rate fold tree's fixed costs
REDUCE_CUTOFF = 280


def ap_of(t, offset, dims, lead=None):
    """Build a raw AP on tile/dram ap `t`: [lead or t.ap[0]] + dims."""
    ap = t.copy()
    first = list(t.ap[0]) if lead is None else list(lead)
    ap.ap = bass_rust.VecI64Pair([first] + [list(d) for d in dims])
    ap.offset = offset
    return ap


def build_nc():
    nc = bacc.Bacc("TRN2", target_bir_lowering=False, debug=False, num_devices=1)
    consts_in = nc.dram_tensor("consts", [B_CORE, CONST_IN], BF16, kind="ExternalInput").ap()
    stops_in = nc.dram_tensor("stops", [B_CORE, STOP_IN], BF16, kind="ExternalInput").ap()
    logs_d = nc.dram_tensor("ecr", [B_CORE, N], F32, kind="ExternalOutput").ap()
    dsum_d = nc.dram_tensor("dsum", [B_CORE], F32, kind="ExternalOutput").ap()

    with tile.TileContext(nc) as tc:
        with tc.tile_pool(name="p", bufs=1) as pool, \
                nc.allow_low_precision(reason="bf16 chart DP by design"):
            banks_t = pool.tile([128, 24 * D], BF16)
            consts_t = pool.tile([128, 16 * D], BF16)
            stops_t = pool.tile([128, 32 * N], BF16)
            zb_t = pool.tile([128, ZB_TOTAL], BF16)
            zf_t = pool.tile([128, ZF_TOTAL], F32)
            banks = banks_t[:]
            consts = consts_t[:]
            stops = stops_t[:]
            zb = zb_t[:]
            zf = zf_t[:]

            v = nc.vector
            sc = nc.scalar

            # ---- load host-precomputed exp-domain constants ----
            nc.sync.dma_start(
                ap_of(stops, 0, [[STOP_IN, G], [1, STOP_IN]]),
                ap_of(stops_in, 0,
                      [[STOP_IN, G], [1, STOP_IN]], lead=[G * STOP_IN, 128]),
            )
            # consts split by row range: step w reads row w only, so later
            # chunks' DMA hides under early DP steps
            for lo, hi in ((0, 2 * N), (2 * N, 5 * N), (5 * N, 10 * N),
                           (10 * N, 18 * N), (18 * N, 29 * N), (29 * N, D)):
                for g in range(G):
                    nc.sync.dma_start(
                        ap_of(consts, 4 * g * D + lo, [[D, 4], [1, hi - lo]]),
                        ap_of(consts_in, g * CONST_IN + lo,
                              [[D, 4], [1, hi - lo]], lead=[G * CONST_IN, 128]),
                    )

            # ---- width-0 init ----
            # The renorm rescale reads full N-wide rows whose tail columns the
            # DP never writes: zero exactly those tails (Pool, no dep with any
            # DP write, so nothing gates the chart loop).
            for d in range(1, RENORM_W + 1):
                # K/C-type tables: row d written on cols [0, N-d)
                nc.gpsimd.memset(
                    ap_of(banks, d * N + (N - d), [[D, 16], [1, d]]), 0.0)
            for r in range(RENORM_W):
                # IR row r written on cols [0, N-r-1); IL on [1, N-r-1]
                nc.gpsimd.memset(
                    ap_of(banks, S_IR * D + r * N + (N - r - 1), [[D, 4], [1, r + 1]]), 0.0)
                if r >= 1:
                    nc.gpsimd.memset(
                        ap_of(banks, S_IL * D + r * N + (N - r), [[D, 4], [1, r]]), 0.0)
            nc.gpsimd.memset(
                ap_of(banks, S_IL * D, [[D, 4], [N, RENORM_W], [1, 1]]), 0.0)
            v.memset(ap_of(zf, ZF_DSUM, [[1, G]]), 0.0)
            # KR[0,:] = KL[0,:] = 1
            v.memset(ap_of(banks, S_KL * D, [[4 * D, 2], [D, 4], [1, N]]), 1.0)
            # CRa[0,i] = exp(stop[i,R,NO]); CLa[0,i] = exp(stop[i,L,NO])
            v.tensor_copy(
                ap_of(banks, S_CRA * D, [[D, 4], [1, N]]),
                ap_of(stops, V_SRNO * N, [[8 * N, 4], [1, N]]),
            )
            v.tensor_copy(
                ap_of(banks, S_CLA * D, [[D, 4], [1, N]]),
                ap_of(stops, V_SLNO * N, [[8 * N, 4], [1, N]]),
            )

            def fold_pair(w, s, t0, count, final0=None, final1=None):
                """In-place fold the two 4-slot halves of P[qg, t, i] over t
                in [t0, t0+count) down to one row at t0, emitting the halves'
                levels interleaved so the independent chains hide each
                other's semaphore latency. final0/final1 redirect each
                half's last fold."""
                sw = s * w
                h = count
                while h > 1:
                    h2 = h // 2
                    hc = h - h2
                    for half, fin in ((0, final0), (1, final1)):
                        base = ZB_P + half * 4 * sw + t0 * s
                        if hc == 1 and fin is not None:
                            out = fin
                        else:
                            out = ap_of(zb, base, [[sw, 4], [s, h2], [1, s]])
                        v.tensor_tensor(
                            out,
                            ap_of(zb, base, [[sw, 4], [s, h2], [1, s]]),
                            ap_of(zb, base + hc * s, [[sw, 4], [s, h2], [1, s]]),
                            OP.add,
                        )
                    h = hc

            # ---- chart DP ----
            for w in range(1, N):
                s = N - w
                sw = s * w
                row = (w - 1) * N + 1
                use_reduce = s * w <= REDUCE_CUTOFF
                # opA products, q=R forward: P[g,t,i] = KR[t,i]*CLa[w-1-t, i+t+1]
                # (t=0 term = CLa[w-1, i+1], the NOCHILD edge, since KR[0]=1)
                v.tensor_tensor(
                    ap_of(zb, ZB_P, [[sw, 4], [s, w], [1, s]]),
                    ap_of(banks, S_KR * D, [[D, 4], [N, w], [1, s]]),
                    ap_of(banks, S_CLA * D + row, [[D, 4], [-40, w], [1, s]]),
                    OP.mult,
                )
                # q=L t-reversed: P[4+g,t',i] = CRa[w-1-t',i]*KL[t', i+w-t']
                # (t'=0 term = CRa[w-1, i], the NOCHILD edge, since KL[0]=1)
                v.tensor_tensor(
                    ap_of(zb, ZB_P + 4 * sw, [[sw, 4], [s, w], [1, s]]),
                    ap_of(banks, S_CRA * D + (w - 1) * N, [[D, 4], [-N, w], [1, s]]),
                    ap_of(banks, S_KL * D + w, [[D, 4], [40, w], [1, s]]),
                    OP.mult,
                )
                if w > 1:
                    # tmp2[q,g,i] = NOCHILD edge * {A0,B0}[w,:] — depends only
                    # on the mults (P[:,0,:]), so it runs under the folds
                    v.tensor_tensor(
                        ap_of(zb, ZB_T2, [[4 * s, 2], [s, 4], [1, s]]),
                        ap_of(zb, ZB_P, [[4 * sw, 2], [sw, 4], [1, s]]),
                        ap_of(consts, C_A0 * D + w * N, [[D, 2], [4 * D, 4], [1, s]]),
                        OP.mult,
                    )
                    if use_reduce:
                        v.tensor_reduce(
                            ap_of(zb, ZB_SS, [[s, 8], [1, s]]),
                            ap_of(zb, ZB_P + s, [[sw, 8], [1, s], [s, w - 1]]),
                            axis=AX.X, op=OP.add,
                        )
                        ss = ap_of(zb, ZB_SS, [[4 * s, 2], [s, 4], [1, s]])
                    else:
                        fold_pair(w, s, 1, w - 1)
                        ss = ap_of(zb, ZB_P + s, [[4 * sw, 2], [sw, 4], [1, s]])
                    # tmp1[q,g,i] = (sum over HASCHILD splits) * {A1,B1}[w,:]
                    v.tensor_tensor(
                        ap_of(zb, ZB_T1, [[4 * s, 2], [s, 4], [1, s]]),
                        ss,
                        ap_of(consts, C_A1 * D + w * N, [[D, 2], [4 * D, 4], [1, s]]),
                        OP.mult,
                    )
                    # IR[w-1, i] = tmp1R + tmp2R; IL[w-1, i+1] = tmp1L + tmp2L
                    # (two ops so each opB mult waits only on its own input)
                    v.tensor_tensor(
                        ap_of(banks, S_IR * D + (w - 1) * N, [[D, 4], [1, s]]),
                        ap_of(zb, ZB_T1, [[s, 4], [1, s]]),
                        ap_of(zb, ZB_T2, [[s, 4], [1, s]]),
                        OP.add,
                    )
                    v.tensor_tensor(
                        ap_of(banks, S_IL * D + (w - 1) * N + 1, [[D, 4], [1, s]]),
                        ap_of(zb, ZB_T1 + 4 * s, [[s, 4], [1, s]]),
                        ap_of(zb, ZB_T2 + 4 * s, [[s, 4], [1, s]]),
                        OP.add,
                    )
                else:
                    # w=1: only the NOCHILD edge exists
                    v.tensor_tensor(
                        ap_of(banks, S_IR * D, [[D, 4], [1, s]]),
                        ap_of(zb, ZB_P, [[sw, 4], [1, s]]),
                        ap_of(consts, C_A0 * D + w * N, [[4 * D, 4], [1, s]]),
                        OP.mult,
                    )
                    v.tensor_tensor(
                        ap_of(banks, S_IL * D + 1, [[D, 4], [1, s]]),
                        ap_of(zb, ZB_P + 4 * sw, [[sw, 4], [1, s]]),
                        ap_of(consts, C_B0 * D + w * N, [[4 * D, 4], [1, s]]),
                        OP.mult,
                    )
                # opB products, half 0 (q=L): P[g,t,i] = CLa[t,i]*IL[w-1-t, i+t+1]
                # half 1 (q=R): P[4+g,t,i] = IR[t,i]*CRa[w-1-t, i+t+1]
                klout = ap_of(banks, S_KL * D + w * N, [[D, 4], [1, s]])
                krout = ap_of(banks, S_KR * D + w * N, [[D, 4], [1, s]])
                if w == 1:
                    v.tensor_tensor(
                        klout,
                        ap_of(banks, S_CLA * D, [[D, 4], [N, 1], [1, s]]),
                        ap_of(banks, S_IL * D + row, [[D, 4], [-40, 1], [1, s]]),
                        OP.mult,
                    )
                    v.tensor_tensor(
                        krout,
                        ap_of(banks, S_IR * D, [[D, 4], [N, 1], [1, s]]),
                        ap_of(banks, S_CRA * D + row, [[D, 4], [-40, 1], [1, s]]),
                        OP.mult,
                    )
                else:
                    v.tensor_tensor(
                        ap_of(zb, ZB_P, [[sw, 4], [s, w], [1, s]]),
                        ap_of(banks, S_CLA * D, [[D, 4], [N, w], [1, s]]),
                        ap_of(banks, S_IL * D + row, [[D, 4], [-40, w], [1, s]]),
                        OP.mult,
                    )
                    v.tensor_tensor(
                        ap_of(zb, ZB_P + 4 * sw, [[sw, 4], [s, w], [1, s]]),
                        ap_of(banks, S_IR * D, [[D, 4], [N, w], [1, s]]),
                        ap_of(banks, S_CRA * D + row, [[D, 4], [-40, w], [1, s]]),
                        OP.mult,
                    )
                    if use_reduce:
                        v.tensor_reduce(
                            ap_of(banks, S_KL * D + w * N, [[D, 8], [1, s]]),
                            ap_of(zb, ZB_P, [[sw, 8], [1, s], [s, w]]),
                            axis=AX.X, op=OP.add,
                        )
                    else:
                        fold_pair(w, s, 0, w, final0=klout, final1=krout)
                # CRa[w,i] = KR[w,i] * sRhas[i]
                v.tensor_tensor(
                    ap_of(banks, S_CRA * D + w * N, [[D, 4], [1, s]]),
                    ap_of(banks, S_KR * D + w * N, [[D, 4], [1, s]]),
                    ap_of(stops, V_SRHAS * N, [[8 * N, 4], [1, s]]),
                    OP.mult,
                )
                # CLa[w,i] = KL[w,i] * sLhas[i+w]
                v.tensor_tensor(
                    ap_of(banks, S_CLA * D + w * N, [[D, 4], [1, s]]),
                    ap_of(banks, S_KL * D + w * N, [[D, 4], [1, s]]),
                    ap_of(stops, V_SLHAS * N + w, [[8 * N, 4], [1, s]]),
                    OP.mult,
                )

                if w == RENORM_W:
                    s0 = N - w
                    # mu[g] = max_i max(KR[w,i], KL[w,i])  (per partition)
                    v.tensor_reduce(
                        ap_of(zf, ZF_M2, [[4, 2], [1, 4]]),
                        ap_of(banks, S_KL * D + w * N, [[4 * D, 2], [D, 4], [1, s0]]),
                        axis=AX.X, op=OP.max,
                    )
                    v.tensor_tensor(
                        ap_of(zf, ZF_MU, [[1, 4]]),
                        ap_of(zf, ZF_M2, [[1, 4]]),
                        ap_of(zf, ZF_M2 + 4, [[1, 4]]),
                        OP.max,
                    )
                    # Ln range on ACT is +-2^64: compute via mu*2^-32
                    v.tensor_scalar_mul(
                        ap_of(zf, ZF_MU, [[1, 4]]), ap_of(zf, ZF_MU, [[1, 4]]), 2.0**-32
                    )
                    v.tensor_scalar_max(
                        ap_of(zf, ZF_MU, [[1, 4]]), ap_of(zf, ZF_MU, [[1, 4]]), 1e-36
                    )
                    sc.activation(
                        ap_of(zf, ZF_LM, [[1, 4]]), ap_of(zf, ZF_MU, [[1, 4]]), AF.Ln
                    )
                    # quantize the per-width shift to delta = -k*ln2 with k
                    # integer, so every rescale factor is an EXACT power of
                    # two (exact in bf16 as well).
                    # kf = round((log(mu*2^-32) + 32 ln2) / (w ln2))
                    v.tensor_scalar(
                        ap_of(zf, ZF_LM, [[1, 4]]), ap_of(zf, ZF_LM, [[1, 4]]),
                        LN2_32, 1.0 / (w * float(np.log(2.0))),
                        OP.add, OP.mult,
                    )
                    v.tensor_scalar(
                        ap_of(zf, ZF_LM, [[1, 4]]), ap_of(zf, ZF_LM, [[1, 4]]),
                        12582912.0, 12582912.0, OP.add, OP.subtract,
                    )
                    # dsum accumulates k (exact small integers)
                    v.tensor_tensor(
                        ap_of(zf, ZF_DSUM, [[1, 4]]),
                        ap_of(zf, ZF_DSUM, [[1, 4]]),
                        ap_of(zf, ZF_LM, [[1, 4]]),
                        OP.add,
                    )
                    # scale2 = 2^-k via exponent bits: (127 - k) << 23
                    v.tensor_scalar(
                        ap_of(zf, ZF_M2, [[1, 4]]), ap_of(zf, ZF_LM, [[1, 4]]),
                        -1.0, 127.0, OP.mult, OP.add,
                    )
                    zi = zf.bitcast(mybir.dt.int32)
                    v.tensor_copy(
                        ap_of(zi, ZF_M2 + 4, [[1, 4]]),
                        ap_of(zf, ZF_M2, [[1, 4]]),
                    )
                    v.tensor_scalar(
                        ap_of(zi, ZF_M2 + 4, [[1, 4]]),
                        ap_of(zi, ZF_M2 + 4, [[1, 4]]),
                        23, None, OP.arith_shift_left,
                    )
                    # M[g, d] = 2^(-k*d): d=0 -> 1, then multiplicative scan
                    v.memset(ap_of(zf, ZF_M, [[42, 4], [1, 1]]), 1.0)
                    for g in range(G):
                        sca = ap_of(zf, ZF_M2 + 4 + g, [[0, 41]])
                        v.tensor_tensor_scan(
                            ap_of(zf, ZF_M + g * 42 + 1, [[1, 41]]),
                            sca, sca, 1.0, OP.mult, OP.bypass,
                        )
                    # expand to Mx[g, d, i] = M[g, d] (packed bf16, exact
                    # powers of two) so the rescales hit the 2x DVE mode
                    v.tensor_copy(
                        ap_of(zb, ZB_MX, [[(w + 2) * N, 4], [N, w + 2], [1, N]]),
                        ap_of(zf, ZF_M, [[42, 4], [1, w + 2], [0, N]]),
                    )
                    # far const rows (needed from w=26 on) rescale on Pool,
                    # overlapping the DVE rescales and the next few widths
                    for g in range(G):
                        tCf = ap_of(consts, 4 * g * D + 26 * N,
                                    [[D, 4], [N, N - 26], [1, N]])
                        nc.gpsimd.tensor_tensor(
                            tCf, tCf,
                            ap_of(zb, ZB_MX + g * (w + 2) * N + N,
                                  [[0, 4], [0, N - 26], [1, N]]),
                            OP.mult,
                        )
                    for g in range(G):
                        mg = ZB_MX + g * (w + 2) * N
                        # natural tables, rows d<=w: scale by 2^(-k*d)
                        tA = ap_of(banks, g * D, [[4 * D, 4], [N, w + 1], [1, N]])
                        v.tensor_tensor(
                            tA, tA,
                            ap_of(zb, mg, [[0, 4], [N, w + 1], [1, N]]),
                            OP.mult,
                        )
                        # IR/IL rows r<=w-1 hold width r+1: scale 2^(-k*(r+1))
                        tI = ap_of(banks, (16 + g) * D, [[4 * D, 2], [N, w], [1, N]])
                        v.tensor_tensor(
                            tI, tI,
                            ap_of(zb, mg + N, [[0, 2], [N, w], [1, N]]),
                            OP.mult,
                        )
                        # near const rows w+1..25: one extra arc factor 2^-k
                        tC = ap_of(consts, 4 * g * D + (w + 1) * N,
                                   [[D, 4], [N, 25 - w], [1, N]])
                        v.tensor_tensor(
                            tC, tC,
                            ap_of(zb, mg + N, [[0, 4], [0, 25 - w], [1, N]]),
                            OP.mult,
                        )

            # ---- extract raw exp-domain CR[0, j] (log on host) ----
            v.tensor_copy(
                ap_of(zf, ZF_CROUT, [[N, 4], [1, N]]),
                ap_of(banks, S_CRA * D, [[D, 4], [N, N]]),
            )
            nc.sync.dma_start(
                ap_of(logs_d, 0, [[N, G], [1, N]], lead=[G * N, 128]),
                ap_of(zf, ZF_CROUT, [[N, G], [1, N]]),
            )
            nc.sync.dma_start(
                ap_of(dsum_d, 0, [[1, G]], lead=[G, 128]),
                ap_of(zf, ZF_DSUM, [[1, G]]),
            )

    nc.compile()
    return nc


_NC_CACHE = {}


def get_nc():
    if "nc" not in _NC_CACHE:
        _NC_CACHE["nc"] = build_nc()
    return _NC_CACHE["nc"]


def make_in_maps(trans_scores, dec_scores):
    t = np.asarray(trans_scores, dtype=np.float32)
    dec = np.asarray(dec_scores, dtype=np.float32)
    B = t.shape[0]
    go = dec[..., 0]                        # [B, n, dir, dv]
    # per-sentence linear pre-shift: each arc factor carries exp(-c0), so a
    # width-w entry is scaled exp(-c0*w); undone on the host at the end.
    tm = np.where(t < -1e8, -np.inf, t).max(axis=3)
    with np.errstate(invalid="ignore"):
        colmax = tm.max(axis=1)             # [B, n] best arc into each child
        proxy = np.nanmean(
            np.where(np.isfinite(colmax), colmax, np.nan)[:, 1:], axis=-1)
    c0 = (proxy + 0.5).astype(np.float32)
    c0 = np.clip(np.nan_to_num(c0), -20.0, 20.0)
    # one exp over trans (NEG -> 0 underflow is intended), then gather diags
    with np.errstate(under="ignore"):
        E = np.exp(t - c0[:, None, None, None])      # [B, n, n, 2]
        ego = np.exp(go)                             # [B, n, 2, 2]
    d_idx, i_idx = np.meshgrid(np.arange(N), np.arange(N), indexing="ij")
    j_idx = np.minimum(i_idx + d_idx, N - 1)
    valid = ((i_idx + d_idx) <= N - 1)[None].astype(np.float32)
    ea = E[:, i_idx, j_idx, :]              # [B, n, n, 2]  trans[i, i+d, v]
    eb = E[:, j_idx, i_idx, :]              # [B, n, n, 2]  trans[i+d, i, v]
    a1 = ea[..., 1] * ego[:, :, 1, 1][:, i_idx] * valid
    a0 = ea[..., 0] * ego[:, :, 1, 0][:, i_idx] * valid
    b1 = eb[..., 1] * ego[:, :, 0, 1][:, j_idx] * valid
    b0 = eb[..., 0] * ego[:, :, 0, 0][:, j_idx] * valid
    consts = np.empty((B, 4, N, N), dtype=np.float32)
    consts[:, 0] = a1
    consts[:, 1] = b1
    consts[:, 2] = a0
    consts[:, 3] = b0
    consts = consts.reshape(B, CONST_IN).astype(ml_dtypes.bfloat16)
    est = np.exp(dec[..., 1])               # [B, n, dir, dv]
    stops = np.empty((B, 8, N), dtype=np.float32)
    stops[:, 0] = ego[:, :, 0, 0]; stops[:, 1] = ego[:, :, 0, 1]
    stops[:, 2] = ego[:, :, 1, 0]; stops[:, 3] = ego[:, :, 1, 1]
    stops[:, 4] = est[:, :, 0, 0]; stops[:, 5] = est[:, :, 0, 1]
    stops[:, 6] = est[:, :, 1, 0]; stops[:, 7] = est[:, :, 1, 1]
    stops = stops.reshape(B, STOP_IN).astype(ml_dtypes.bfloat16)
    in_maps = []
    for c in range(NCORES):
        sl = slice(c * B_CORE, (c + 1) * B_CORE)
        in_maps.append({
            "consts": consts[sl],
            "stops": stops[sl],
        })
    return in_maps, c0


L0_HOST = 5  # sentences with len <= L0_HOST are computed exactly on the host


def _host_short_ll(trans, dec, lens):
    """Exact f64 LL for short sentences via the inside DP truncated to
    positions 0..L0_HOST (spans of a length-l sentence live within [0, l])."""
    n = L0_HOST + 1
    t = np.asarray(trans)[:, :n, :n, :].astype(np.float64)
    dc = np.asarray(dec)[:, :n].astype(np.float64)
    B = t.shape[0]
    go = dc[..., 0]
    stop = dc[..., 1]
    NEG = -1e9
    IR = np.full((B, n, n), NEG)
    IL = np.full((B, n, n), NEG)
    KR = np.full((B, n, n), NEG)
    KL = np.full((B, n, n), NEG)
    dg = np.arange(n)
    KR[:, dg, dg] = 0.0
    KL[:, dg, dg] = 0.0
    CR = np.full((B, n, n), NEG)
    CL = np.full((B, n, n), NEG)
    CR[:, dg, dg] = stop[:, :, 1, 0]
    CL[:, dg, dg] = stop[:, :, 0, 0]
    goR = go[:, :, 1, :]
    goL = go[:, :, 0, :]

    def lse(x):
        m = x.max(axis=-1, keepdims=True)
        return np.squeeze(m, -1) + np.log(np.exp(x - m).sum(axis=-1))

    for w in range(1, n):
        s = n - w
        i = np.arange(s)[:, None]
        tt = np.arange(w)[None, :]
        j = i + w
        k = i + tt
        vR = (tt > 0).astype(np.int64)
        ir = lse(KR[:, i, k] + goR[:, i, vR] + t[:, i, j, np.minimum(vR, 1)]
                 + CL[:, k + 1, j])
        vL = (tt < w - 1).astype(np.int64)
        il = lse(CR[:, i, k] + KL[:, k + 1, j] + goL[:, j, vL]
                 + t[:, j, i, np.minimum(vL, 1)])
        i1 = np.arange(s)
        IR[:, i1, i1 + w] = ir
        IL[:, i1, i1 + w] = il
        kr = lse(IR[:, i, i + 1 + tt] + CR[:, i + 1 + tt, j])
        kl = lse(CL[:, i, i + tt] + IL[:, i + tt, j])
        KR[:, i1, i1 + w] = kr
        KL[:, i1, i1 + w] = kl
        CR[:, i1, i1 + w] = kr + stop[:, i1, 1, 1]
        CL[:, i1, i1 + w] = kl + stop[:, i1 + w, 0, 1]

    return CR[np.arange(B), 0, lens].astype(np.float32)


def assemble(results, len_array, c0):
    ln = np.asarray(len_array).astype(np.int64)
    c0 = np.asarray(c0).astype(np.float64)
    out = np.empty(len(ln), dtype=np.float32)
    for c, res in enumerate(results):
        ecr = res["ecr"].reshape(B_CORE, N).astype(np.float64)
        dsum = res["dsum"].reshape(B_CORE).astype(np.float64)
        lc = ln[c * B_CORE:(c + 1) * B_CORE]
        idx = np.arange(B_CORE)
        with np.errstate(divide="ignore"):
            out[c * B_CORE:(c + 1) * B_CORE] = (
                np.log(ecr[idx, lc]) + dsum * np.log(2.0) * lc
                + c0[c * B_CORE:(c + 1) * B_CORE] * lc
            ).astype(np.float32)
    return out


def kernel(trans_scores, dec_scores, len_array):
    from concourse.bass_utils import run_bass_kernel_spmd

    nc = get_nc()
    in_maps, c0 = make_in_maps(trans_scores, dec_scores)
    res = run_bass_kernel_spmd(nc, in_maps, core_ids=list(range(NCORES)))
    out = assemble(res.results, len_array, c0)
    lens = np.asarray(len_array).astype(np.int64)
    short = lens <= L0_HOST
    if short.any():
        out[short] = _host_short_ll(
            np.asarray(trans_scores)[short], np.asarray(dec_scores)[short],
            lens[short])
    return out


# revision 16
# speedup vs baseline: 1.9476x; 1.0464x over previous
"""DMV inside algorithm (Eisner chart DP, logsumexp semiring) on Trainium2.

Strategy
--------
Pure data parallelism over the batch: 4096 sentences -> 8 cores x 512.
Per core: ONE run of 512 sentences laid out as [128 SBUF partitions] x
[G=4 sentence groups in the free dim], all chart tables in bf16.

The DP runs in the *exp domain* (no per-split transcendentals): tables hold
exp(score). Each width-w update is one fused strided multiply (products
P[qg,t,i]) followed by an in-place folding tree of adds that reduces over
the split dim t. Everything iterates [qg, t, i] with i innermost and
stride 1, which (with bf16) hits the DVE 2x packed-16-bit mode; the tree
of tensor_tensor adds also runs at 2x, unlike InstTensorReduce which gets
no fast mode.

Tables are stored *diagonal-packed*: Xd[d*41 + i] = X[i, i+d], making
every gather in the width-w recurrence a constant-stride access pattern.
IR/IL are stored with row r holding width r+1 (IL column-shifted by +1)
so all four quantities' gathers share one AP shape.

Numerics: scale composes linearly in span width (every width-w entry has
exactly w arcs), so one on-device renormalization at w=20 multiplies row
d by an exact power of two 2^(-k*d) (k integer per sentence), keeping
everything in range (bf16 range == f32 range). k is returned per sentence
and undone on the host: LL = log(CR[0,len]) + k*ln2*len + c0*len.
"""

import os

os.environ.setdefault("JAX_PLATFORMS", "cpu")

import numpy as np
import ml_dtypes

import concourse.bass as bass  # noqa: F401  (registers engine classes)
import concourse.tile as tile
import bass_rust
from concourse import bacc, mybir

F32 = mybir.dt.float32
BF16 = mybir.dt.bfloat16
AF = mybir.ActivationFunctionType
OP = mybir.AluOpType
AX = mybir.AxisListType

N = 41              # fake_len (ROOT at 0)
D = 1681            # table pitch: N*N elements
G = 4               # sentence groups per partition
NCORES = 8
B_CORE = 128 * G    # 512
CONST_IN = 4 * D    # host sends 4 pre-exponentiated tables/sentence (bf16)
STOP_IN = 8 * N     # host sends 8 exp'd stop/go vectors/sentence (bf16)
RENORM_W = 20

# banks tile (bf16): 24 diag-packed tables, slot k at offset k*D.
# Since every DP multiply is a per-direction 4-slot op, no table needs a
# second copy; all paired-access strides stay within 16*D (ISA AP steps
# are 16-bit).
#   KL g0..g3: 0..3    KR: 4..7    CRa: 8..11   CLa: 12..15
#   IR: 16..19         IL: 20..23  (IL stored col+1)
S_KL, S_KR, S_CRA, S_CLA, S_IR, S_IL = 0, 4, 8, 12, 16, 20
# consts tile (bf16): 4 per-arc tables, g-major: offset (4*g + C)*D
# (A0/B0 kept verbatim, not as differences: all-positive arithmetic so
# bf16 never hits catastrophic cancellation)
C_A1, C_B1, C_A0, C_B0 = range(4)
# stops tile (bf16): 32 vectors of 41, offset (g*8 + v)*41
V_GL0, V_GL1, V_GR0, V_GR1, V_SLNO, V_SLHAS, V_SRNO, V_SRHAS = range(8)

# bf16 scratch tile element offsets
ZB_P = 0          # 3360: products [qg, t, i] (qg-stride = s*w, varies by w)
ZB_T1 = 3360      # 328: tmp1 [q, g, i]
ZB_T2 = 3688      # 328
ZB_SS = 4016      # 328: reduce-path sums [q, g, i]
ZB_MX = 4344      # 4*22*41: expanded renorm multiplier Mx[g, d, i] = 2^(-k_g*d)
ZB_TOTAL = 7952

# f32 scratch tile element offsets
ZF_M2 = 0         # 8
ZF_MU = 8         # 4
ZF_LM = 12        # 4 (reused for k)
ZF_M = 16         # 4*42: renorm multiplier table [g, 42]
ZF_CROUT = 184    # 4*41
ZF_DSUM = 348     # 4
ZF_TOTAL = 352

LN2_32 = 32.0 * float(np.log(2.0))

# below this many (elements per fold-chain * levels) a single TensorReduce
# (1x rate but one instruction) beats the 2x-rate fold tree's fixed costs
REDUCE_CUTOFF = 0  # folds win at every width (interleaving hides their latency)


def ap_of(t, offset, dims, lead=None):
    """Build a raw AP on tile/dram ap `t`: [lead or t.ap[0]] + dims."""
    ap = t.copy()
    first = list(t.ap[0]) if lead is None else list(lead)
    ap.ap = bass_rust.VecI64Pair([first] + [list(d) for d in dims])
    ap.offset = offset
    return ap


def build_nc():
    nc = bacc.Bacc("TRN2", target_bir_lowering=False, debug=False, num_devices=1)
    consts_in = nc.dram_tensor("consts", [B_CORE, CONST_IN], BF16, kind="ExternalInput").ap()
    stops_in = nc.dram_tensor("stops", [B_CORE, STOP_IN], BF16, kind="ExternalInput").ap()
    logs_d = nc.dram_tensor("ecr", [B_CORE, N], F32, kind="ExternalOutput").ap()
    dsum_d = nc.dram_tensor("dsum", [B_CORE], F32, kind="ExternalOutput").ap()

    with tile.TileContext(nc) as tc:
        with tc.tile_pool(name="p", bufs=1) as pool, \
                nc.allow_low_precision(reason="bf16 chart DP by design"):
            banks_t = pool.tile([128, 24 * D], BF16)
            consts_t = pool.tile([128, 16 * D], BF16)
            stops_t = pool.tile([128, 32 * N], BF16)
            zb_t = pool.tile([128, ZB_TOTAL], BF16)
            zf_t = pool.tile([128, ZF_TOTAL], F32)
            banks = banks_t[:]
            consts = consts_t[:]
            stops = stops_t[:]
            zb = zb_t[:]
            zf = zf_t[:]

            v = nc.vector
            sc = nc.scalar

            # ---- load host-precomputed exp-domain constants ----
            nc.sync.dma_start(
                ap_of(stops, 0, [[STOP_IN, G], [1, STOP_IN]]),
                ap_of(stops_in, 0,
                      [[STOP_IN, G], [1, STOP_IN]], lead=[G * STOP_IN, 128]),
            )
            # consts split by row range: step w reads row w only, so later
            # chunks' DMA hides under early DP steps
            for lo, hi in ((0, 2 * N), (2 * N, 5 * N), (5 * N, 10 * N),
                           (10 * N, 18 * N), (18 * N, 29 * N), (29 * N, D)):
                for g in range(G):
                    nc.sync.dma_start(
                        ap_of(consts, 4 * g * D + lo, [[D, 4], [1, hi - lo]]),
                        ap_of(consts_in, g * CONST_IN + lo,
                              [[D, 4], [1, hi - lo]], lead=[G * CONST_IN, 128]),
                    )

            # ---- width-0 init ----
            # The renorm rescale reads full N-wide rows whose tail columns the
            # DP never writes: zero exactly those tails (Pool, no dep with any
            # DP write, so nothing gates the chart loop).
            for d in range(1, RENORM_W + 1):
                # K/C-type tables: row d written on cols [0, N-d)
                nc.gpsimd.memset(
                    ap_of(banks, d * N + (N - d), [[D, 16], [1, d]]), 0.0)
            for r in range(RENORM_W):
                # IR row r written on cols [0, N-r-1); IL on [1, N-r-1]
                nc.gpsimd.memset(
                    ap_of(banks, S_IR * D + r * N + (N - r - 1), [[D, 4], [1, r + 1]]), 0.0)
                if r >= 1:
                    nc.gpsimd.memset(
                        ap_of(banks, S_IL * D + r * N + (N - r), [[D, 4], [1, r]]), 0.0)
            nc.gpsimd.memset(
                ap_of(banks, S_IL * D, [[D, 4], [N, RENORM_W], [1, 1]]), 0.0)
            v.memset(ap_of(zf, ZF_DSUM, [[1, G]]), 0.0)
            # KR[0,:] = KL[0,:] = 1
            v.memset(ap_of(banks, S_KL * D, [[4 * D, 2], [D, 4], [1, N]]), 1.0)
            # CRa[0,i] = exp(stop[i,R,NO]); CLa[0,i] = exp(stop[i,L,NO])
            v.tensor_copy(
                ap_of(banks, S_CRA * D, [[D, 4], [1, N]]),
                ap_of(stops, V_SRNO * N, [[8 * N, 4], [1, N]]),
            )
            v.tensor_copy(
                ap_of(banks, S_CLA * D, [[D, 4], [1, N]]),
                ap_of(stops, V_SLNO * N, [[8 * N, 4], [1, N]]),
            )

            def fold_pair(w, s, t0, count, final0=None, final1=None):
                """In-place fold the two 4-slot halves of P[qg, t, i] over t
                in [t0, t0+count) down to one row at t0, emitting the halves'
                levels interleaved so the independent chains hide each
                other's semaphore latency. final0/final1 redirect each
                half's last fold."""
                sw = s * w
                h = count
                while h > 1:
                    h2 = h // 2
                    hc = h - h2
                    for half, fin in ((0, final0), (1, final1)):
                        base = ZB_P + half * 4 * sw + t0 * s
                        if hc == 1 and fin is not None:
                            out = fin
                        else:
                            out = ap_of(zb, base, [[sw, 4], [s, h2], [1, s]])
                        v.tensor_tensor(
                            out,
                            ap_of(zb, base, [[sw, 4], [s, h2], [1, s]]),
                            ap_of(zb, base + hc * s, [[sw, 4], [s, h2], [1, s]]),
                            OP.add,
                        )
                    h = hc

            # ---- chart DP ----
            for w in range(1, N):
                s = N - w
                sw = s * w
                row = (w - 1) * N + 1
                # opA: NOCHILD edges (t=0 for q=R, t=w-1 for q=L) have a
                # trivial K-factor of 1, so products cover only the w-1
                # HASCHILD splits; the edges are read straight from banks.
                # tmp2[q,g,i] = NOCHILD edge * {A0,B0}[w,:]
                v.tensor_tensor(
                    ap_of(zb, ZB_T2, [[4 * s, 2], [s, 4], [1, s]]),
                    ap_of(banks, S_CLA * D + (w - 1) * N + 1,
                          [[-4 * D - 1, 2], [D, 4], [1, s]]),
                    ap_of(consts, C_A0 * D + w * N, [[D, 2], [4 * D, 4], [1, s]]),
                    OP.mult,
                )
                if w > 1:
                    sw1 = s * (w - 1)
                    # q=R forward, t in [1,w): P[g,t,i] = KR[t,i]*CLa[w-1-t, i+t+1]
                    v.tensor_tensor(
                        ap_of(zb, ZB_P, [[sw1, 4], [s, w - 1], [1, s]]),
                        ap_of(banks, S_KR * D + N, [[D, 4], [N, w - 1], [1, s]]),
                        ap_of(banks, S_CLA * D + row - 40, [[D, 4], [-40, w - 1], [1, s]]),
                        OP.mult,
                    )
                    # q=L t-reversed, t' in [1,w): P[4+g,t',i] = CRa[w-1-t',i]*KL[t', i+w-t']
                    v.tensor_tensor(
                        ap_of(zb, ZB_P + 4 * sw1, [[sw1, 4], [s, w - 1], [1, s]]),
                        ap_of(banks, S_CRA * D + (w - 2) * N, [[D, 4], [-N, w - 1], [1, s]]),
                        ap_of(banks, S_KL * D + N + w - 1, [[D, 4], [40, w - 1], [1, s]]),
                        OP.mult,
                    )
                    fold_pair(w - 1, s, 0, w - 1)
                    # tmp1[q,g,i] = (sum over HASCHILD splits) * {A1,B1}[w,:]
                    v.tensor_tensor(
                        ap_of(zb, ZB_T1, [[4 * s, 2], [s, 4], [1, s]]),
                        ap_of(zb, ZB_P, [[4 * sw1, 2], [sw1, 4], [1, s]]),
                        ap_of(consts, C_A1 * D + w * N, [[D, 2], [4 * D, 4], [1, s]]),
                        OP.mult,
                    )
                    # IR[w-1, i] = tmp1R + tmp2R; IL[w-1, i+1] = tmp1L + tmp2L
                    # (two ops so each opB mult waits only on its own input)
                    v.tensor_tensor(
                        ap_of(banks, S_IR * D + (w - 1) * N, [[D, 4], [1, s]]),
                        ap_of(zb, ZB_T1, [[s, 4], [1, s]]),
                        ap_of(zb, ZB_T2, [[s, 4], [1, s]]),
                        OP.add,
                    )
                    v.tensor_tensor(
                        ap_of(banks, S_IL * D + (w - 1) * N + 1, [[D, 4], [1, s]]),
                        ap_of(zb, ZB_T1 + 4 * s, [[s, 4], [1, s]]),
                        ap_of(zb, ZB_T2 + 4 * s, [[s, 4], [1, s]]),
                        OP.add,
                    )
                else:
                    # w=1: only the NOCHILD edge exists: IR/IL row 0 = tmp2
                    v.tensor_copy(
                        ap_of(banks, S_IR * D, [[4 * D + 1, 2], [D, 4], [1, s]]),
                        ap_of(zb, ZB_T2, [[4 * s, 2], [s, 4], [1, s]]),
                    )
                # opB products, half 0 (q=L): P[g,t,i] = CLa[t,i]*IL[w-1-t, i+t+1]
                # half 1 (q=R): P[4+g,t,i] = IR[t,i]*CRa[w-1-t, i+t+1]
                klout = ap_of(banks, S_KL * D + w * N, [[D, 4], [1, s]])
                krout = ap_of(banks, S_KR * D + w * N, [[D, 4], [1, s]])
                if w == 1:
                    v.tensor_tensor(
                        klout,
                        ap_of(banks, S_CLA * D, [[D, 4], [N, 1], [1, s]]),
                        ap_of(banks, S_IL * D + row, [[D, 4], [-40, 1], [1, s]]),
                        OP.mult,
                    )
                    v.tensor_tensor(
                        krout,
                        ap_of(banks, S_IR * D, [[D, 4], [N, 1], [1, s]]),
                        ap_of(banks, S_CRA * D + row, [[D, 4], [-40, 1], [1, s]]),
                        OP.mult,
                    )
                else:
                    v.tensor_tensor(
                        ap_of(zb, ZB_P, [[sw, 4], [s, w], [1, s]]),
                        ap_of(banks, S_CLA * D, [[D, 4], [N, w], [1, s]]),
                        ap_of(banks, S_IL * D + row, [[D, 4], [-40, w], [1, s]]),
                        OP.mult,
                    )
                    v.tensor_tensor(
                        ap_of(zb, ZB_P + 4 * sw, [[sw, 4], [s, w], [1, s]]),
                        ap_of(banks, S_IR * D, [[D, 4], [N, w], [1, s]]),
                        ap_of(banks, S_CRA * D + row, [[D, 4], [-40, w], [1, s]]),
                        OP.mult,
                    )
                    fold_pair(w, s, 0, w, final0=klout, final1=krout)
                # CRa[w,i] = KR[w,i]*sRhas[i]; CLa[w,i] = KL[w,i]*sLhas[i+w]
                v.tensor_tensor(
                    ap_of(banks, S_CRA * D + w * N, [[4 * D, 2], [D, 4], [1, s]]),
                    ap_of(banks, S_KR * D + w * N, [[-4 * D, 2], [D, 4], [1, s]]),
                    ap_of(stops, V_SRHAS * N, [[w - 2 * N, 2], [8 * N, 4], [1, s]]),
                    OP.mult,
                )

                if w == RENORM_W:
                    s0 = N - w
                    # mu[g] = max_i max(KR[w,i], KL[w,i])  (per partition)
                    v.tensor_reduce(
                        ap_of(zf, ZF_M2, [[4, 2], [1, 4]]),
                        ap_of(banks, S_KL * D + w * N, [[4 * D, 2], [D, 4], [1, s0]]),
                        axis=AX.X, op=OP.max,
                    )
                    v.tensor_tensor(
                        ap_of(zf, ZF_MU, [[1, 4]]),
                        ap_of(zf, ZF_M2, [[1, 4]]),
                        ap_of(zf, ZF_M2 + 4, [[1, 4]]),
                        OP.max,
                    )
                    # Ln range on ACT is +-2^64: compute via mu*2^-32
                    v.tensor_scalar_mul(
                        ap_of(zf, ZF_MU, [[1, 4]]), ap_of(zf, ZF_MU, [[1, 4]]), 2.0**-32
                    )
                    v.tensor_scalar_max(
                        ap_of(zf, ZF_MU, [[1, 4]]), ap_of(zf, ZF_MU, [[1, 4]]), 1e-36
                    )
                    sc.activation(
                        ap_of(zf, ZF_LM, [[1, 4]]), ap_of(zf, ZF_MU, [[1, 4]]), AF.Ln
                    )
                    # quantize the per-width shift to delta = -k*ln2 with k
                    # integer, so every rescale factor is an EXACT power of
                    # two (exact in bf16 as well).
                    # kf = round((log(mu*2^-32) + 32 ln2) / (w ln2))
                    v.tensor_scalar(
                        ap_of(zf, ZF_LM, [[1, 4]]), ap_of(zf, ZF_LM, [[1, 4]]),
                        LN2_32, 1.0 / (w * float(np.log(2.0))),
                        OP.add, OP.mult,
                    )
                    v.tensor_scalar(
                        ap_of(zf, ZF_LM, [[1, 4]]), ap_of(zf, ZF_LM, [[1, 4]]),
                        12582912.0, 12582912.0, OP.add, OP.subtract,
                    )
                    # dsum accumulates k (exact small integers)
                    v.tensor_tensor(
                        ap_of(zf, ZF_DSUM, [[1, 4]]),
                        ap_of(zf, ZF_DSUM, [[1, 4]]),
                        ap_of(zf, ZF_LM, [[1, 4]]),
                        OP.add,
                    )
                    # scale2 = 2^-k via exponent bits: (127 - k) << 23
                    v.tensor_scalar(
                        ap_of(zf, ZF_M2, [[1, 4]]), ap_of(zf, ZF_LM, [[1, 4]]),
                        -1.0, 127.0, OP.mult, OP.add,
                    )
                    zi = zf.bitcast(mybir.dt.int32)
                    v.tensor_copy(
                        ap_of(zi, ZF_M2 + 4, [[1, 4]]),
                        ap_of(zf, ZF_M2, [[1, 4]]),
                    )
                    v.tensor_scalar(
                        ap_of(zi, ZF_M2 + 4, [[1, 4]]),
                        ap_of(zi, ZF_M2 + 4, [[1, 4]]),
                        23, None, OP.arith_shift_left,
                    )
                    # M[g, d] = 2^(-k*d): d=0 -> 1, then multiplicative scan
                    v.memset(ap_of(zf, ZF_M, [[42, 4], [1, 1]]), 1.0)
                    for g in range(G):
                        sca = ap_of(zf, ZF_M2 + 4 + g, [[0, 41]])
                        v.tensor_tensor_scan(
                            ap_of(zf, ZF_M + g * 42 + 1, [[1, 41]]),
                            sca, sca, 1.0, OP.mult, OP.bypass,
                        )
                    # expand to Mx[g, d, i] = M[g, d] (packed bf16, exact
                    # powers of two) so the rescales hit the 2x DVE mode
                    v.tensor_copy(
                        ap_of(zb, ZB_MX, [[(w + 2) * N, 4], [N, w + 2], [1, N]]),
                        ap_of(zf, ZF_M, [[42, 4], [1, w + 2], [0, N]]),
                    )
                    # far const rows (needed from w=26 on) rescale on Pool,
                    # overlapping the DVE rescales and the next few widths
                    for g in range(G):
                        tCf = ap_of(consts, 4 * g * D + 26 * N,
                                    [[D, 4], [N, N - 26], [1, N]])
                        nc.gpsimd.tensor_tensor(
                            tCf, tCf,
                            ap_of(zb, ZB_MX + g * (w + 2) * N + N,
                                  [[0, 4], [0, N - 26], [1, N]]),
                            OP.mult,
                        )
                    for g in range(G):
                        mg = ZB_MX + g * (w + 2) * N
                        # natural tables, rows d<=w: scale by 2^(-k*d)
                        tA = ap_of(banks, g * D, [[4 * D, 4], [N, w + 1], [1, N]])
                        v.tensor_tensor(
                            tA, tA,
                            ap_of(zb, mg, [[0, 4], [N, w + 1], [1, N]]),
                            OP.mult,
                        )
                        # IR/IL rows r<=w-1 hold width r+1: scale 2^(-k*(r+1))
                        tI = ap_of(banks, (16 + g) * D, [[4 * D, 2], [N, w], [1, N]])
                        v.tensor_tensor(
                            tI, tI,
                            ap_of(zb, mg + N, [[0, 2], [N, w], [1, N]]),
                            OP.mult,
                        )
                        # near const rows w+1..25: one extra arc factor 2^-k
                        tC = ap_of(consts, 4 * g * D + (w + 1) * N,
                                   [[D, 4], [N, 25 - w], [1, N]])
                        v.tensor_tensor(
                            tC, tC,
                            ap_of(zb, mg + N, [[0, 4], [0, 25 - w], [1, N]]),
                            OP.mult,
                        )

            # ---- extract raw exp-domain CR[0, j] (log on host) ----
            v.tensor_copy(
                ap_of(zf, ZF_CROUT, [[N, 4], [1, N]]),
                ap_of(banks, S_CRA * D, [[D, 4], [N, N]]),
            )
            nc.sync.dma_start(
                ap_of(logs_d, 0, [[N, G], [1, N]], lead=[G * N, 128]),
                ap_of(zf, ZF_CROUT, [[N, G], [1, N]]),
            )
            nc.sync.dma_start(
                ap_of(dsum_d, 0, [[1, G]], lead=[G, 128]),
                ap_of(zf, ZF_DSUM, [[1, G]]),
            )

    nc.compile()
    return nc


_NC_CACHE = {}


def get_nc():
    if "nc" not in _NC_CACHE:
        _NC_CACHE["nc"] = build_nc()
    return _NC_CACHE["nc"]


def make_in_maps(trans_scores, dec_scores):
    t = np.asarray(trans_scores, dtype=np.float32)
    dec = np.asarray(dec_scores, dtype=np.float32)
    B = t.shape[0]
    go = dec[..., 0]                        # [B, n, dir, dv]
    # per-sentence linear pre-shift: each arc factor carries exp(-c0), so a
    # width-w entry is scaled exp(-c0*w); undone on the host at the end.
    tm = np.where(t < -1e8, -np.inf, t).max(axis=3)
    with np.errstate(invalid="ignore"):
        colmax = tm.max(axis=1)             # [B, n] best arc into each child
        proxy = np.nanmean(
            np.where(np.isfinite(colmax), colmax, np.nan)[:, 1:], axis=-1)
    c0 = (proxy + 0.5).astype(np.float32)
    c0 = np.clip(np.nan_to_num(c0), -20.0, 20.0)
    # one exp over trans (NEG -> 0 underflow is intended), then gather diags
    with np.errstate(under="ignore"):
        E = np.exp(t - c0[:, None, None, None])      # [B, n, n, 2]
        ego = np.exp(go)                             # [B, n, 2, 2]
    d_idx, i_idx = np.meshgrid(np.arange(N), np.arange(N), indexing="ij")
    j_idx = np.minimum(i_idx + d_idx, N - 1)
    valid = ((i_idx + d_idx) <= N - 1)[None].astype(np.float32)
    ea = E[:, i_idx, j_idx, :]              # [B, n, n, 2]  trans[i, i+d, v]
    eb = E[:, j_idx, i_idx, :]              # [B, n, n, 2]  trans[i+d, i, v]
    a1 = ea[..., 1] * ego[:, :, 1, 1][:, i_idx] * valid
    a0 = ea[..., 0] * ego[:, :, 1, 0][:, i_idx] * valid
    b1 = eb[..., 1] * ego[:, :, 0, 1][:, j_idx] * valid
    b0 = eb[..., 0] * ego[:, :, 0, 0][:, j_idx] * valid
    consts = np.empty((B, 4, N, N), dtype=np.float32)
    consts[:, 0] = a1
    consts[:, 1] = b1
    consts[:, 2] = a0
    consts[:, 3] = b0
    consts = consts.reshape(B, CONST_IN).astype(ml_dtypes.bfloat16)
    est = np.exp(dec[..., 1])               # [B, n, dir, dv]
    stops = np.empty((B, 8, N), dtype=np.float32)
    stops[:, 0] = ego[:, :, 0, 0]; stops[:, 1] = ego[:, :, 0, 1]
    stops[:, 2] = ego[:, :, 1, 0]; stops[:, 3] = ego[:, :, 1, 1]
    stops[:, 4] = est[:, :, 0, 0]; stops[:, 5] = est[:, :, 0, 1]
    stops[:, 6] = est[:, :, 1, 0]; stops[:, 7] = est[:, :, 1, 1]
    stops = stops.reshape(B, STOP_IN).astype(ml_dtypes.bfloat16)
    in_maps = []
    for c in range(NCORES):
        sl = slice(c * B_CORE, (c + 1) * B_CORE)
        in_maps.append({
            "consts": consts[sl],
            "stops": stops[sl],
        })
    return in_maps, c0


L0_HOST = 5  # sentences with len <= L0_HOST are computed exactly on the host


def _host_short_ll(trans, dec, lens):
    """Exact f64 LL for short sentences via the inside DP truncated to
    positions 0..L0_HOST (spans of a length-l sentence live within [0, l])."""
    n = L0_HOST + 1
    t = np.asarray(trans)[:, :n, :n, :].astype(np.float64)
    dc = np.asarray(dec)[:, :n].astype(np.float64)
    B = t.shape[0]
    go = dc[..., 0]
    stop = dc[..., 1]
    NEG = -1e9
    IR = np.full((B, n, n), NEG)
    IL = np.full((B, n, n), NEG)
    KR = np.full((B, n, n), NEG)
    KL = np.full((B, n, n), NEG)
    dg = np.arange(n)
    KR[:, dg, dg] = 0.0
    KL[:, dg, dg] = 0.0
    CR = np.full((B, n, n), NEG)
    CL = np.full((B, n, n), NEG)
    CR[:, dg, dg] = stop[:, :, 1, 0]
    CL[:, dg, dg] = stop[:, :, 0, 0]
    goR = go[:, :, 1, :]
    goL = go[:, :, 0, :]

    def lse(x):
        m = x.max(axis=-1, keepdims=True)
        return np.squeeze(m, -1) + np.log(np.exp(x - m).sum(axis=-1))

    for w in range(1, n):
        s = n - w
        i = np.arange(s)[:, None]
        tt = np.arange(w)[None, :]
        j = i + w
        k = i + tt
        vR = (tt > 0).astype(np.int64)
        ir = lse(KR[:, i, k] + goR[:, i, vR] + t[:, i, j, np.minimum(vR, 1)]
                 + CL[:, k + 1, j])
        vL = (tt < w - 1).astype(np.int64)
        il = lse(CR[:, i, k] + KL[:, k + 1, j] + goL[:, j, vL]
                 + t[:, j, i, np.minimum(vL, 1)])
        i1 = np.arange(s)
        IR[:, i1, i1 + w] = ir
        IL[:, i1, i1 + w] = il
        kr = lse(IR[:, i, i + 1 + tt] + CR[:, i + 1 + tt, j])
        kl = lse(CL[:, i, i + tt] + IL[:, i + tt, j])
        KR[:, i1, i1 + w] = kr
        KL[:, i1, i1 + w] = kl
        CR[:, i1, i1 + w] = kr + stop[:, i1, 1, 1]
        CL[:, i1, i1 + w] = kl + stop[:, i1 + w, 0, 1]

    return CR[np.arange(B), 0, lens].astype(np.float32)


def assemble(results, len_array, c0):
    ln = np.asarray(len_array).astype(np.int64)
    c0 = np.asarray(c0).astype(np.float64)
    out = np.empty(len(ln), dtype=np.float32)
    for c, res in enumerate(results):
        ecr = res["ecr"].reshape(B_CORE, N).astype(np.float64)
        dsum = res["dsum"].reshape(B_CORE).astype(np.float64)
        lc = ln[c * B_CORE:(c + 1) * B_CORE]
        idx = np.arange(B_CORE)
        with np.errstate(divide="ignore"):
            out[c * B_CORE:(c + 1) * B_CORE] = (
                np.log(ecr[idx, lc]) + dsum * np.log(2.0) * lc
                + c0[c * B_CORE:(c + 1) * B_CORE] * lc
            ).astype(np.float32)
    return out


def kernel(trans_scores, dec_scores, len_array):
    from concourse.bass_utils import run_bass_kernel_spmd

    nc = get_nc()
    in_maps, c0 = make_in_maps(trans_scores, dec_scores)
    res = run_bass_kernel_spmd(nc, in_maps, core_ids=list(range(NCORES)))
    out = assemble(res.results, len_array, c0)
    lens = np.asarray(len_array).astype(np.int64)
    short = lens <= L0_HOST
    if short.any():
        out[short] = _host_short_ll(
            np.asarray(trans_scores)[short], np.asarray(dec_scores)[short],
            lens[short])
    return out


# revision 17
# speedup vs baseline: 1.9482x; 1.0003x over previous
"""DMV inside algorithm (Eisner chart DP, logsumexp semiring) on Trainium2.

Strategy
--------
Pure data parallelism over the batch: 4096 sentences -> 8 cores x 512.
Per core: ONE run of 512 sentences laid out as [128 SBUF partitions] x
[G=4 sentence groups in the free dim], all chart tables in bf16.

The DP runs in the *exp domain* (no per-split transcendentals): tables hold
exp(score). Each width-w update is one fused strided multiply (products
P[qg,t,i]) followed by an in-place folding tree of adds that reduces over
the split dim t. Everything iterates [qg, t, i] with i innermost and
stride 1, which (with bf16) hits the DVE 2x packed-16-bit mode; the tree
of tensor_tensor adds also runs at 2x, unlike InstTensorReduce which gets
no fast mode.

Tables are stored *diagonal-packed*: Xd[d*41 + i] = X[i, i+d], making
every gather in the width-w recurrence a constant-stride access pattern.
IR/IL are stored with row r holding width r+1 (IL column-shifted by +1)
so all four quantities' gathers share one AP shape.

Numerics: scale composes linearly in span width (every width-w entry has
exactly w arcs), so one on-device renormalization at w=20 multiplies row
d by an exact power of two 2^(-k*d) (k integer per sentence), keeping
everything in range (bf16 range == f32 range). k is returned per sentence
and undone on the host: LL = log(CR[0,len]) + k*ln2*len + c0*len.
"""

import os

os.environ.setdefault("JAX_PLATFORMS", "cpu")

import numpy as np
import ml_dtypes

import concourse.bass as bass  # noqa: F401  (registers engine classes)
import concourse.tile as tile
import bass_rust
from concourse import bacc, mybir

F32 = mybir.dt.float32
BF16 = mybir.dt.bfloat16
AF = mybir.ActivationFunctionType
OP = mybir.AluOpType
AX = mybir.AxisListType

N = 41              # fake_len (ROOT at 0)
D = 1681            # table pitch: N*N elements
G = 4               # sentence groups per partition
NCORES = 8
B_CORE = 128 * G    # 512
CONST_IN = 4 * D    # host sends 4 pre-exponentiated tables/sentence (bf16)
STOP_IN = 4 * N     # host sends 4 exp'd stop vectors/sentence (bf16)
RENORM_W = 20

# banks tile (bf16): 24 diag-packed tables, slot k at offset k*D.
# Since every DP multiply is a per-direction 4-slot op, no table needs a
# second copy; all paired-access strides stay within 16*D (ISA AP steps
# are 16-bit).
#   KL g0..g3: 0..3    KR: 4..7    CRa: 8..11   CLa: 12..15
#   IR: 16..19         IL: 20..23  (IL stored col+1)
S_KL, S_KR, S_CRA, S_CLA, S_IR, S_IL = 0, 4, 8, 12, 16, 20
# consts tile (bf16): 4 per-arc tables, g-major: offset (4*g + C)*D
# (A0/B0 kept verbatim, not as differences: all-positive arithmetic so
# bf16 never hits catastrophic cancellation)
C_A1, C_B1, C_A0, C_B0 = range(4)
# stops tile (bf16): 16 vectors of 41, offset (g*4 + v)*41
V_SLNO, V_SLHAS, V_SRNO, V_SRHAS = range(4)

# bf16 scratch tile element offsets
ZB_P = 0          # 3360: products [qg, t, i] (qg-stride = s*w, varies by w)
ZB_T1 = 3360      # 328: tmp1 [q, g, i]
ZB_T2 = 3688      # 328
ZB_SS = 4016      # 328: reduce-path sums [q, g, i]
ZB_MX = 4344      # 4*22*41: expanded renorm multiplier Mx[g, d, i] = 2^(-k_g*d)
ZB_TOTAL = 7952

# f32 scratch tile element offsets
ZF_M2 = 0         # 8
ZF_MU = 8         # 4
ZF_LM = 12        # 4 (reused for k)
ZF_M = 16         # 4*42: renorm multiplier table [g, 42]
ZF_CROUT = 184    # 4*41
ZF_DSUM = 348     # 4
ZF_TOTAL = 352

LN2_32 = 32.0 * float(np.log(2.0))

# below this many (elements per fold-chain * levels) a single TensorReduce
# (1x rate but one instruction) beats the 2x-rate fold tree's fixed costs
REDUCE_CUTOFF = 0  # folds win at every width (interleaving hides their latency)


def ap_of(t, offset, dims, lead=None):
    """Build a raw AP on tile/dram ap `t`: [lead or t.ap[0]] + dims."""
    ap = t.copy()
    first = list(t.ap[0]) if lead is None else list(lead)
    ap.ap = bass_rust.VecI64Pair([first] + [list(d) for d in dims])
    ap.offset = offset
    return ap


def build_nc():
    nc = bacc.Bacc("TRN2", target_bir_lowering=False, debug=False, num_devices=1)
    consts_in = nc.dram_tensor("consts", [B_CORE, CONST_IN], BF16, kind="ExternalInput").ap()
    stops_in = nc.dram_tensor("stops", [B_CORE, STOP_IN], BF16, kind="ExternalInput").ap()
    logs_d = nc.dram_tensor("ecr", [B_CORE, N], F32, kind="ExternalOutput").ap()
    dsum_d = nc.dram_tensor("dsum", [B_CORE], F32, kind="ExternalOutput").ap()

    with tile.TileContext(nc) as tc:
        with tc.tile_pool(name="p", bufs=1) as pool, \
                nc.allow_low_precision(reason="bf16 chart DP by design"):
            banks_t = pool.tile([128, 24 * D], BF16)
            consts_t = pool.tile([128, 16 * D], BF16)
            stops_t = pool.tile([128, 16 * N], BF16)
            zb_t = pool.tile([128, ZB_TOTAL], BF16)
            zf_t = pool.tile([128, ZF_TOTAL], F32)
            banks = banks_t[:]
            consts = consts_t[:]
            stops = stops_t[:]
            zb = zb_t[:]
            zf = zf_t[:]

            v = nc.vector
            sc = nc.scalar

            # ---- load host-precomputed exp-domain constants ----
            nc.sync.dma_start(
                ap_of(stops, 0, [[STOP_IN, G], [1, STOP_IN]]),
                ap_of(stops_in, 0,
                      [[STOP_IN, G], [1, STOP_IN]], lead=[G * STOP_IN, 128]),
            )
            # consts split by row range: step w reads row w only, so later
            # chunks' DMA hides under early DP steps
            for lo, hi in ((0, 2 * N), (2 * N, 5 * N), (5 * N, 10 * N),
                           (10 * N, 18 * N), (18 * N, 29 * N), (29 * N, D)):
                nc.sync.dma_start(
                    ap_of(consts, lo, [[4 * D, G], [D, 4], [1, hi - lo]]),
                    ap_of(consts_in, lo, [[CONST_IN, G], [D, 4], [1, hi - lo]],
                          lead=[G * CONST_IN, 128]),
                )

            # ---- width-0 init ----
            # The renorm rescale reads full N-wide rows whose tail columns the
            # DP never writes: zero exactly those tails (Pool, no dep with any
            # DP write, so nothing gates the chart loop).
            for d in range(1, RENORM_W + 1):
                # K/C-type tables: row d written on cols [0, N-d)
                nc.gpsimd.memset(
                    ap_of(banks, d * N + (N - d), [[D, 16], [1, d]]), 0.0)
            for r in range(RENORM_W):
                # IR row r written on cols [0, N-r-1); IL on [1, N-r-1]
                nc.gpsimd.memset(
                    ap_of(banks, S_IR * D + r * N + (N - r - 1), [[D, 4], [1, r + 1]]), 0.0)
                if r >= 1:
                    nc.gpsimd.memset(
                        ap_of(banks, S_IL * D + r * N + (N - r), [[D, 4], [1, r]]), 0.0)
            nc.gpsimd.memset(
                ap_of(banks, S_IL * D, [[D, 4], [N, RENORM_W], [1, 1]]), 0.0)
            v.memset(ap_of(zf, ZF_DSUM, [[1, G]]), 0.0)
            # KR[0,:] = KL[0,:] = 1
            v.memset(ap_of(banks, S_KL * D, [[4 * D, 2], [D, 4], [1, N]]), 1.0)
            # CRa[0,i] = exp(stop[i,R,NO]); CLa[0,i] = exp(stop[i,L,NO])
            v.tensor_copy(
                ap_of(banks, S_CRA * D, [[D, 4], [1, N]]),
                ap_of(stops, V_SRNO * N, [[4 * N, 4], [1, N]]),
            )
            v.tensor_copy(
                ap_of(banks, S_CLA * D, [[D, 4], [1, N]]),
                ap_of(stops, V_SLNO * N, [[4 * N, 4], [1, N]]),
            )

            def fold_pair(w, s, t0, count, final0=None, final1=None):
                """In-place fold the two 4-slot halves of P[qg, t, i] over t
                in [t0, t0+count) down to one row at t0, emitting the halves'
                levels interleaved so the independent chains hide each
                other's semaphore latency. final0/final1 redirect each
                half's last fold."""
                sw = s * w
                h = count
                while h > 1:
                    h2 = h // 2
                    hc = h - h2
                    for half, fin in ((0, final0), (1, final1)):
                        base = ZB_P + half * 4 * sw + t0 * s
                        if hc == 1 and fin is not None:
                            out = fin
                        else:
                            out = ap_of(zb, base, [[sw, 4], [s, h2], [1, s]])
                        v.tensor_tensor(
                            out,
                            ap_of(zb, base, [[sw, 4], [s, h2], [1, s]]),
                            ap_of(zb, base + hc * s, [[sw, 4], [s, h2], [1, s]]),
                            OP.add,
                        )
                    h = hc

            # ---- chart DP ----
            for w in range(1, N):
                s = N - w
                sw = s * w
                row = (w - 1) * N + 1
                # opA: NOCHILD edges (t=0 for q=R, t=w-1 for q=L) have a
                # trivial K-factor of 1, so products cover only the w-1
                # HASCHILD splits; the edges are read straight from banks.
                # tmp2[q,g,i] = NOCHILD edge * {A0,B0}[w,:]
                v.tensor_tensor(
                    ap_of(zb, ZB_T2, [[4 * s, 2], [s, 4], [1, s]]),
                    ap_of(banks, S_CLA * D + (w - 1) * N + 1,
                          [[-4 * D - 1, 2], [D, 4], [1, s]]),
                    ap_of(consts, C_A0 * D + w * N, [[D, 2], [4 * D, 4], [1, s]]),
                    OP.mult,
                )
                if w > 1:
                    sw1 = s * (w - 1)
                    # q=R forward, t in [1,w): P[g,t,i] = KR[t,i]*CLa[w-1-t, i+t+1]
                    v.tensor_tensor(
                        ap_of(zb, ZB_P, [[sw1, 4], [s, w - 1], [1, s]]),
                        ap_of(banks, S_KR * D + N, [[D, 4], [N, w - 1], [1, s]]),
                        ap_of(banks, S_CLA * D + row - 40, [[D, 4], [-40, w - 1], [1, s]]),
                        OP.mult,
                    )
                    # q=L t-reversed, t' in [1,w): P[4+g,t',i] = CRa[w-1-t',i]*KL[t', i+w-t']
                    v.tensor_tensor(
                        ap_of(zb, ZB_P + 4 * sw1, [[sw1, 4], [s, w - 1], [1, s]]),
                        ap_of(banks, S_CRA * D + (w - 2) * N, [[D, 4], [-N, w - 1], [1, s]]),
                        ap_of(banks, S_KL * D + N + w - 1, [[D, 4], [40, w - 1], [1, s]]),
                        OP.mult,
                    )
                    fold_pair(w - 1, s, 0, w - 1)
                    # tmp1[q,g,i] = (sum over HASCHILD splits) * {A1,B1}[w,:]
                    v.tensor_tensor(
                        ap_of(zb, ZB_T1, [[4 * s, 2], [s, 4], [1, s]]),
                        ap_of(zb, ZB_P, [[4 * sw1, 2], [sw1, 4], [1, s]]),
                        ap_of(consts, C_A1 * D + w * N, [[D, 2], [4 * D, 4], [1, s]]),
                        OP.mult,
                    )
                    # IR[w-1, i] = tmp1R + tmp2R; IL[w-1, i+1] = tmp1L + tmp2L
                    # (two ops so each opB mult waits only on its own input)
                    v.tensor_tensor(
                        ap_of(banks, S_IR * D + (w - 1) * N, [[D, 4], [1, s]]),
                        ap_of(zb, ZB_T1, [[s, 4], [1, s]]),
                        ap_of(zb, ZB_T2, [[s, 4], [1, s]]),
                        OP.add,
                    )
                    v.tensor_tensor(
                        ap_of(banks, S_IL * D + (w - 1) * N + 1, [[D, 4], [1, s]]),
                        ap_of(zb, ZB_T1 + 4 * s, [[s, 4], [1, s]]),
                        ap_of(zb, ZB_T2 + 4 * s, [[s, 4], [1, s]]),
                        OP.add,
                    )
                else:
                    # w=1: only the NOCHILD edge exists: IR/IL row 0 = tmp2
                    v.tensor_copy(
                        ap_of(banks, S_IR * D, [[4 * D + 1, 2], [D, 4], [1, s]]),
                        ap_of(zb, ZB_T2, [[4 * s, 2], [s, 4], [1, s]]),
                    )
                # opB products, half 0 (q=L): P[g,t,i] = CLa[t,i]*IL[w-1-t, i+t+1]
                # half 1 (q=R): P[4+g,t,i] = IR[t,i]*CRa[w-1-t, i+t+1]
                klout = ap_of(banks, S_KL * D + w * N, [[D, 4], [1, s]])
                krout = ap_of(banks, S_KR * D + w * N, [[D, 4], [1, s]])
                if w == 1:
                    v.tensor_tensor(
                        klout,
                        ap_of(banks, S_CLA * D, [[D, 4], [N, 1], [1, s]]),
                        ap_of(banks, S_IL * D + row, [[D, 4], [-40, 1], [1, s]]),
                        OP.mult,
                    )
                    v.tensor_tensor(
                        krout,
                        ap_of(banks, S_IR * D, [[D, 4], [N, 1], [1, s]]),
                        ap_of(banks, S_CRA * D + row, [[D, 4], [-40, 1], [1, s]]),
                        OP.mult,
                    )
                else:
                    v.tensor_tensor(
                        ap_of(zb, ZB_P, [[sw, 4], [s, w], [1, s]]),
                        ap_of(banks, S_CLA * D, [[D, 4], [N, w], [1, s]]),
                        ap_of(banks, S_IL * D + row, [[D, 4], [-40, w], [1, s]]),
                        OP.mult,
                    )
                    v.tensor_tensor(
                        ap_of(zb, ZB_P + 4 * sw, [[sw, 4], [s, w], [1, s]]),
                        ap_of(banks, S_IR * D, [[D, 4], [N, w], [1, s]]),
                        ap_of(banks, S_CRA * D + row, [[D, 4], [-40, w], [1, s]]),
                        OP.mult,
                    )
                    fold_pair(w, s, 0, w, final0=klout, final1=krout)
                # CRa[w,i] = KR[w,i]*sRhas[i]; CLa[w,i] = KL[w,i]*sLhas[i+w]
                v.tensor_tensor(
                    ap_of(banks, S_CRA * D + w * N, [[4 * D, 2], [D, 4], [1, s]]),
                    ap_of(banks, S_KR * D + w * N, [[-4 * D, 2], [D, 4], [1, s]]),
                    ap_of(stops, V_SRHAS * N, [[w - 2 * N, 2], [4 * N, 4], [1, s]]),
                    OP.mult,
                )

                if w == RENORM_W:
                    s0 = N - w
                    # mu[g] = max_i max(KR[w,i], KL[w,i])  (per partition)
                    v.tensor_reduce(
                        ap_of(zf, ZF_M2, [[4, 2], [1, 4]]),
                        ap_of(banks, S_KL * D + w * N, [[4 * D, 2], [D, 4], [1, s0]]),
                        axis=AX.X, op=OP.max,
                    )
                    v.tensor_tensor(
                        ap_of(zf, ZF_MU, [[1, 4]]),
                        ap_of(zf, ZF_M2, [[1, 4]]),
                        ap_of(zf, ZF_M2 + 4, [[1, 4]]),
                        OP.max,
                    )
                    # Ln range on ACT is +-2^64: compute via mu*2^-32
                    v.tensor_scalar_mul(
                        ap_of(zf, ZF_MU, [[1, 4]]), ap_of(zf, ZF_MU, [[1, 4]]), 2.0**-32
                    )
                    v.tensor_scalar_max(
                        ap_of(zf, ZF_MU, [[1, 4]]), ap_of(zf, ZF_MU, [[1, 4]]), 1e-36
                    )
                    sc.activation(
                        ap_of(zf, ZF_LM, [[1, 4]]), ap_of(zf, ZF_MU, [[1, 4]]), AF.Ln
                    )
                    # quantize the per-width shift to delta = -k*ln2 with k
                    # integer, so every rescale factor is an EXACT power of
                    # two (exact in bf16 as well).
                    # kf = round((log(mu*2^-32) + 32 ln2) / (w ln2))
                    v.tensor_scalar(
                        ap_of(zf, ZF_LM, [[1, 4]]), ap_of(zf, ZF_LM, [[1, 4]]),
                        LN2_32, 1.0 / (w * float(np.log(2.0))),
                        OP.add, OP.mult,
                    )
                    v.tensor_scalar(
                        ap_of(zf, ZF_LM, [[1, 4]]), ap_of(zf, ZF_LM, [[1, 4]]),
                        12582912.0, 12582912.0, OP.add, OP.subtract,
                    )
                    # dsum accumulates k (exact small integers)
                    v.tensor_tensor(
                        ap_of(zf, ZF_DSUM, [[1, 4]]),
                        ap_of(zf, ZF_DSUM, [[1, 4]]),
                        ap_of(zf, ZF_LM, [[1, 4]]),
                        OP.add,
                    )
                    # scale2 = 2^-k via exponent bits: (127 - k) << 23
                    v.tensor_scalar(
                        ap_of(zf, ZF_M2, [[1, 4]]), ap_of(zf, ZF_LM, [[1, 4]]),
                        -1.0, 127.0, OP.mult, OP.add,
                    )
                    zi = zf.bitcast(mybir.dt.int32)
                    v.tensor_copy(
                        ap_of(zi, ZF_M2 + 4, [[1, 4]]),
                        ap_of(zf, ZF_M2, [[1, 4]]),
                    )
                    v.tensor_scalar(
                        ap_of(zi, ZF_M2 + 4, [[1, 4]]),
                        ap_of(zi, ZF_M2 + 4, [[1, 4]]),
                        23, None, OP.arith_shift_left,
                    )
                    # M[g, d] = 2^(-k*d): d=0 -> 1, then multiplicative scan
                    v.memset(ap_of(zf, ZF_M, [[42, 4], [1, 1]]), 1.0)
                    for g in range(G):
                        sca = ap_of(zf, ZF_M2 + 4 + g, [[0, 41]])
                        v.tensor_tensor_scan(
                            ap_of(zf, ZF_M + g * 42 + 1, [[1, 41]]),
                            sca, sca, 1.0, OP.mult, OP.bypass,
                        )
                    # expand to Mx[g, d, i] = M[g, d] (packed bf16, exact
                    # powers of two) so the rescales hit the 2x DVE mode
                    v.tensor_copy(
                        ap_of(zb, ZB_MX, [[(w + 2) * N, 4], [N, w + 2], [1, N]]),
                        ap_of(zf, ZF_M, [[42, 4], [1, w + 2], [0, N]]),
                    )
                    # far const rows (needed from w=26 on) rescale on Pool,
                    # overlapping the DVE rescales and the next few widths
                    for g in range(G):
                        tCf = ap_of(consts, 4 * g * D + 26 * N,
                                    [[D, 4], [N, N - 26], [1, N]])
                        nc.gpsimd.tensor_tensor(
                            tCf, tCf,
                            ap_of(zb, ZB_MX + g * (w + 2) * N + N,
                                  [[0, 4], [0, N - 26], [1, N]]),
                            OP.mult,
                        )
                    for g in range(G):
                        mg = ZB_MX + g * (w + 2) * N
                        # natural tables, rows d<=w: scale by 2^(-k*d)
                        tA = ap_of(banks, g * D, [[4 * D, 4], [N, w + 1], [1, N]])
                        v.tensor_tensor(
                            tA, tA,
                            ap_of(zb, mg, [[0, 4], [N, w + 1], [1, N]]),
                            OP.mult,
                        )
                        # IR/IL rows r<=w-1 hold width r+1: scale 2^(-k*(r+1))
                        tI = ap_of(banks, (16 + g) * D, [[4 * D, 2], [N, w], [1, N]])
                        v.tensor_tensor(
                            tI, tI,
                            ap_of(zb, mg + N, [[0, 2], [N, w], [1, N]]),
                            OP.mult,
                        )
                        # near const rows w+1..25: one extra arc factor 2^-k
                        tC = ap_of(consts, 4 * g * D + (w + 1) * N,
                                   [[D, 4], [N, 25 - w], [1, N]])
                        v.tensor_tensor(
                            tC, tC,
                            ap_of(zb, mg + N, [[0, 4], [0, 25 - w], [1, N]]),
                            OP.mult,
                        )

            # ---- extract raw exp-domain CR[0, j] (log on host) ----
            v.tensor_copy(
                ap_of(zf, ZF_CROUT, [[N, 4], [1, N]]),
                ap_of(banks, S_CRA * D, [[D, 4], [N, N]]),
            )
            nc.sync.dma_start(
                ap_of(logs_d, 0, [[N, G], [1, N]], lead=[G * N, 128]),
                ap_of(zf, ZF_CROUT, [[N, G], [1, N]]),
            )
            nc.sync.dma_start(
                ap_of(dsum_d, 0, [[1, G]], lead=[G, 128]),
                ap_of(zf, ZF_DSUM, [[1, G]]),
            )

    nc.compile()
    return nc


_NC_CACHE = {}


def get_nc():
    if "nc" not in _NC_CACHE:
        _NC_CACHE["nc"] = build_nc()
    return _NC_CACHE["nc"]


def make_in_maps(trans_scores, dec_scores):
    t = np.asarray(trans_scores, dtype=np.float32)
    dec = np.asarray(dec_scores, dtype=np.float32)
    B = t.shape[0]
    go = dec[..., 0]                        # [B, n, dir, dv]
    # per-sentence linear pre-shift: each arc factor carries exp(-c0), so a
    # width-w entry is scaled exp(-c0*w); undone on the host at the end.
    tm = np.where(t < -1e8, -np.inf, t).max(axis=3)
    with np.errstate(invalid="ignore"):
        colmax = tm.max(axis=1)             # [B, n] best arc into each child
        proxy = np.nanmean(
            np.where(np.isfinite(colmax), colmax, np.nan)[:, 1:], axis=-1)
    c0 = (proxy + 0.5).astype(np.float32)
    c0 = np.clip(np.nan_to_num(c0), -20.0, 20.0)
    # one exp over trans (NEG -> 0 underflow is intended), then gather diags
    with np.errstate(under="ignore"):
        E = np.exp(t - c0[:, None, None, None])      # [B, n, n, 2]
        ego = np.exp(go)                             # [B, n, 2, 2]
    d_idx, i_idx = np.meshgrid(np.arange(N), np.arange(N), indexing="ij")
    j_idx = np.minimum(i_idx + d_idx, N - 1)
    valid = ((i_idx + d_idx) <= N - 1)[None].astype(np.float32)
    ea = E[:, i_idx, j_idx, :]              # [B, n, n, 2]  trans[i, i+d, v]
    eb = E[:, j_idx, i_idx, :]              # [B, n, n, 2]  trans[i+d, i, v]
    a1 = ea[..., 1] * ego[:, :, 1, 1][:, i_idx] * valid
    a0 = ea[..., 0] * ego[:, :, 1, 0][:, i_idx] * valid
    b1 = eb[..., 1] * ego[:, :, 0, 1][:, j_idx] * valid
    b0 = eb[..., 0] * ego[:, :, 0, 0][:, j_idx] * valid
    consts = np.empty((B, 4, N, N), dtype=np.float32)
    consts[:, 0] = a1
    consts[:, 1] = b1
    consts[:, 2] = a0
    consts[:, 3] = b0
    consts = consts.reshape(B, CONST_IN).astype(ml_dtypes.bfloat16)
    est = np.exp(dec[..., 1])               # [B, n, dir, dv]
    stops = np.empty((B, 4, N), dtype=np.float32)
    stops[:, 0] = est[:, :, 0, 0]; stops[:, 1] = est[:, :, 0, 1]
    stops[:, 2] = est[:, :, 1, 0]; stops[:, 3] = est[:, :, 1, 1]
    stops = stops.reshape(B, STOP_IN).astype(ml_dtypes.bfloat16)
    in_maps = []
    for c in range(NCORES):
        sl = slice(c * B_CORE, (c + 1) * B_CORE)
        in_maps.append({
            "consts": consts[sl],
            "stops": stops[sl],
        })
    return in_maps, c0


L0_HOST = 5  # sentences with len <= L0_HOST are computed exactly on the host


def _host_short_ll(trans, dec, lens):
    """Exact f64 LL for short sentences via the inside DP truncated to
    positions 0..L0_HOST (spans of a length-l sentence live within [0, l])."""
    n = L0_HOST + 1
    t = np.asarray(trans)[:, :n, :n, :].astype(np.float64)
    dc = np.asarray(dec)[:, :n].astype(np.float64)
    B = t.shape[0]
    go = dc[..., 0]
    stop = dc[..., 1]
    NEG = -1e9
    IR = np.full((B, n, n), NEG)
    IL = np.full((B, n, n), NEG)
    KR = np.full((B, n, n), NEG)
    KL = np.full((B, n, n), NEG)
    dg = np.arange(n)
    KR[:, dg, dg] = 0.0
    KL[:, dg, dg] = 0.0
    CR = np.full((B, n, n), NEG)
    CL = np.full((B, n, n), NEG)
    CR[:, dg, dg] = stop[:, :, 1, 0]
    CL[:, dg, dg] = stop[:, :, 0, 0]
    goR = go[:, :, 1, :]
    goL = go[:, :, 0, :]

    def lse(x):
        m = x.max(axis=-1, keepdims=True)
        return np.squeeze(m, -1) + np.log(np.exp(x - m).sum(axis=-1))

    for w in range(1, n):
        s = n - w
        i = np.arange(s)[:, None]
        tt = np.arange(w)[None, :]
        j = i + w
        k = i + tt
        vR = (tt > 0).astype(np.int64)
        ir = lse(KR[:, i, k] + goR[:, i, vR] + t[:, i, j, np.minimum(vR, 1)]
                 + CL[:, k + 1, j])
        vL = (tt < w - 1).astype(np.int64)
        il = lse(CR[:, i, k] + KL[:, k + 1, j] + goL[:, j, vL]
                 + t[:, j, i, np.minimum(vL, 1)])
        i1 = np.arange(s)
        IR[:, i1, i1 + w] = ir
        IL[:, i1, i1 + w] = il
        kr = lse(IR[:, i, i + 1 + tt] + CR[:, i + 1 + tt, j])
        kl = lse(CL[:, i, i + tt] + IL[:, i + tt, j])
        KR[:, i1, i1 + w] = kr
        KL[:, i1, i1 + w] = kl
        CR[:, i1, i1 + w] = kr + stop[:, i1, 1, 1]
        CL[:, i1, i1 + w] = kl + stop[:, i1 + w, 0, 1]

    return CR[np.arange(B), 0, lens].astype(np.float32)


def assemble(results, len_array, c0):
    ln = np.asarray(len_array).astype(np.int64)
    c0 = np.asarray(c0).astype(np.float64)
    out = np.empty(len(ln), dtype=np.float32)
    for c, res in enumerate(results):
        ecr = res["ecr"].reshape(B_CORE, N).astype(np.float64)
        dsum = res["dsum"].reshape(B_CORE).astype(np.float64)
        lc = ln[c * B_CORE:(c + 1) * B_CORE]
        idx = np.arange(B_CORE)
        with np.errstate(divide="ignore"):
            out[c * B_CORE:(c + 1) * B_CORE] = (
                np.log(ecr[idx, lc]) + dsum * np.log(2.0) * lc
                + c0[c * B_CORE:(c + 1) * B_CORE] * lc
            ).astype(np.float32)
    return out


def kernel(trans_scores, dec_scores, len_array):
    from concourse.bass_utils import run_bass_kernel_spmd

    nc = get_nc()
    in_maps, c0 = make_in_maps(trans_scores, dec_scores)
    res = run_bass_kernel_spmd(nc, in_maps, core_ids=list(range(NCORES)))
    out = assemble(res.results, len_array, c0)
    lens = np.asarray(len_array).astype(np.int64)
    short = lens <= L0_HOST
    if short.any():
        out[short] = _host_short_ll(
            np.asarray(trans_scores)[short], np.asarray(dec_scores)[short],
            lens[short])
    return out


# revision 18
# speedup vs baseline: 1.9489x; 1.0003x over previous
"""DMV inside algorithm (Eisner chart DP, logsumexp semiring) on Trainium2.

Strategy
--------
Pure data parallelism over the batch: 4096 sentences -> 8 cores x 512.
Per core: ONE run of 512 sentences laid out as [128 SBUF partitions] x
[G=4 sentence groups in the free dim], all chart tables in bf16.

The DP runs in the *exp domain* (no per-split transcendentals): tables hold
exp(score). Each width-w update is one fused strided multiply (products
P[qg,t,i]) followed by an in-place folding tree of adds that reduces over
the split dim t. Everything iterates [qg, t, i] with i innermost and
stride 1, which (with bf16) hits the DVE 2x packed-16-bit mode; the tree
of tensor_tensor adds also runs at 2x, unlike InstTensorReduce which gets
no fast mode.

Tables are stored *diagonal-packed*: Xd[d*41 + i] = X[i, i+d], making
every gather in the width-w recurrence a constant-stride access pattern.
IR/IL are stored with row r holding width r+1 (IL column-shifted by +1)
so all four quantities' gathers share one AP shape.

Numerics: scale composes linearly in span width (every width-w entry has
exactly w arcs), so one on-device renormalization at w=20 multiplies row
d by an exact power of two 2^(-k*d) (k integer per sentence), keeping
everything in range (bf16 range == f32 range). k is returned per sentence
and undone on the host: LL = log(CR[0,len]) + k*ln2*len + c0*len.
"""

import os

os.environ.setdefault("JAX_PLATFORMS", "cpu")

import numpy as np
import ml_dtypes

import concourse.bass as bass  # noqa: F401  (registers engine classes)
import concourse.tile as tile
import bass_rust
from concourse import bacc, mybir

F32 = mybir.dt.float32
BF16 = mybir.dt.bfloat16
AF = mybir.ActivationFunctionType
OP = mybir.AluOpType
AX = mybir.AxisListType

N = 41              # fake_len (ROOT at 0)
D = 1681            # table pitch: N*N elements
G = 4               # sentence groups per partition
NCORES = 8
B_CORE = 128 * G    # 512
CONST_IN = 4 * D    # host sends 4 pre-exponentiated tables/sentence (bf16)
STOP_IN = 4 * N     # host sends 4 exp'd stop vectors/sentence (bf16)
RENORM_W = 20

# banks tile (bf16): 24 diag-packed tables, slot k at offset k*D.
# Since every DP multiply is a per-direction 4-slot op, no table needs a
# second copy; all paired-access strides stay within 16*D (ISA AP steps
# are 16-bit).
#   KL g0..g3: 0..3    KR: 4..7    CRa: 8..11   CLa: 12..15
#   IR: 16..19         IL: 20..23  (IL stored col+1)
S_KL, S_KR, S_CRA, S_CLA, S_IR, S_IL = 0, 4, 8, 12, 16, 20
# consts tile (bf16): 4 per-arc tables, g-major: offset (4*g + C)*D
# (A0/B0 kept verbatim, not as differences: all-positive arithmetic so
# bf16 never hits catastrophic cancellation)
C_A1, C_B1, C_A0, C_B0 = range(4)
# stops tile (bf16): 16 vectors of 41, offset (g*4 + v)*41
V_SLNO, V_SLHAS, V_SRNO, V_SRHAS = range(4)

# bf16 scratch tile element offsets
ZB_P = 0          # 3360: products [qg, t, i] (qg-stride = s*w, varies by w)
ZB_T1 = 3360      # 328: tmp1 [q, g, i]
ZB_T2 = 3688      # 328
ZB_SS = 4016      # 328: reduce-path sums [q, g, i]
ZB_MX = 4344      # 4*22*41: expanded renorm multiplier Mx[g, d, i] = 2^(-k_g*d)
ZB_TOTAL = 7952

# f32 scratch tile element offsets
ZF_M2 = 0         # 8
ZF_MU = 8         # 4
ZF_LM = 12        # 4 (reused for k)
ZF_M = 16         # 4*42: renorm multiplier table [g, 42]
ZF_CROUT = 184    # 4*41
ZF_DSUM = 348     # 4
ZF_TOTAL = 352

LN2_32 = 32.0 * float(np.log(2.0))

# below this many (elements per fold-chain * levels) a single TensorReduce
# (1x rate but one instruction) beats the 2x-rate fold tree's fixed costs
REDUCE_CUTOFF = 0  # folds win at every width (interleaving hides their latency)


def ap_of(t, offset, dims, lead=None):
    """Build a raw AP on tile/dram ap `t`: [lead or t.ap[0]] + dims."""
    ap = t.copy()
    first = list(t.ap[0]) if lead is None else list(lead)
    ap.ap = bass_rust.VecI64Pair([first] + [list(d) for d in dims])
    ap.offset = offset
    return ap


def build_nc():
    nc = bacc.Bacc("TRN2", target_bir_lowering=False, debug=False, num_devices=1)
    consts_in = nc.dram_tensor("consts", [B_CORE, CONST_IN], BF16, kind="ExternalInput").ap()
    stops_in = nc.dram_tensor("stops", [B_CORE, STOP_IN], BF16, kind="ExternalInput").ap()
    logs_d = nc.dram_tensor("ecr", [B_CORE, N], F32, kind="ExternalOutput").ap()
    dsum_d = nc.dram_tensor("dsum", [B_CORE], F32, kind="ExternalOutput").ap()

    with tile.TileContext(nc) as tc:
        with tc.tile_pool(name="p", bufs=1) as pool, \
                nc.allow_low_precision(reason="bf16 chart DP by design"):
            banks_t = pool.tile([128, 24 * D], BF16)
            consts_t = pool.tile([128, 16 * D], BF16)
            stops_t = pool.tile([128, 16 * N], BF16)
            zb_t = pool.tile([128, ZB_TOTAL], BF16)
            zf_t = pool.tile([128, ZF_TOTAL], F32)
            banks = banks_t[:]
            consts = consts_t[:]
            stops = stops_t[:]
            zb = zb_t[:]
            zf = zf_t[:]

            v = nc.vector
            sc = nc.scalar

            # ---- load host-precomputed exp-domain constants ----
            nc.sync.dma_start(
                ap_of(stops, 0, [[STOP_IN, G], [1, STOP_IN]]),
                ap_of(stops_in, 0,
                      [[STOP_IN, G], [1, STOP_IN]], lead=[G * STOP_IN, 128]),
            )
            # consts split by row range: step w reads row w only, so later
            # chunks' DMA hides under early DP steps
            # chunks of >=7 rows: a chunk of one table is (hi-lo)*82B
            # contiguous in DRAM, and >=512B descriptors avoid the small-
            # descriptor DMA penalty
            for lo, hi in ((0, 7 * N), (7 * N, 14 * N), (14 * N, 22 * N),
                           (22 * N, 31 * N), (31 * N, D)):
                nc.sync.dma_start(
                    ap_of(consts, lo, [[4 * D, G], [D, 4], [1, hi - lo]]),
                    ap_of(consts_in, lo, [[CONST_IN, G], [D, 4], [1, hi - lo]],
                          lead=[G * CONST_IN, 128]),
                )

            # ---- width-0 init ----
            # The renorm rescale reads full N-wide rows whose tail columns the
            # DP never writes: zero exactly those tails (Pool, no dep with any
            # DP write, so nothing gates the chart loop).
            for d in range(1, RENORM_W + 1):
                # K/C-type tables: row d written on cols [0, N-d)
                nc.gpsimd.memset(
                    ap_of(banks, d * N + (N - d), [[D, 16], [1, d]]), 0.0)
            for r in range(RENORM_W):
                # IR row r written on cols [0, N-r-1); IL on [1, N-r-1]
                nc.gpsimd.memset(
                    ap_of(banks, S_IR * D + r * N + (N - r - 1), [[D, 4], [1, r + 1]]), 0.0)
                if r >= 1:
                    nc.gpsimd.memset(
                        ap_of(banks, S_IL * D + r * N + (N - r), [[D, 4], [1, r]]), 0.0)
            nc.gpsimd.memset(
                ap_of(banks, S_IL * D, [[D, 4], [N, RENORM_W], [1, 1]]), 0.0)
            v.memset(ap_of(zf, ZF_DSUM, [[1, G]]), 0.0)
            # KR[0,:] = KL[0,:] = 1
            v.memset(ap_of(banks, S_KL * D, [[4 * D, 2], [D, 4], [1, N]]), 1.0)
            # CRa[0,i] = exp(stop[i,R,NO]); CLa[0,i] = exp(stop[i,L,NO])
            v.tensor_copy(
                ap_of(banks, S_CRA * D, [[D, 4], [1, N]]),
                ap_of(stops, V_SRNO * N, [[4 * N, 4], [1, N]]),
            )
            v.tensor_copy(
                ap_of(banks, S_CLA * D, [[D, 4], [1, N]]),
                ap_of(stops, V_SLNO * N, [[4 * N, 4], [1, N]]),
            )

            def fold_pair(w, s, t0, count, final0=None, final1=None):
                """In-place fold the two 4-slot halves of P[qg, t, i] over t
                in [t0, t0+count) down to one row at t0, emitting the halves'
                levels interleaved so the independent chains hide each
                other's semaphore latency. final0/final1 redirect each
                half's last fold."""
                sw = s * w
                h = count
                while h > 1:
                    h2 = h // 2
                    hc = h - h2
                    for half, fin in ((0, final0), (1, final1)):
                        base = ZB_P + half * 4 * sw + t0 * s
                        if hc == 1 and fin is not None:
                            out = fin
                        else:
                            out = ap_of(zb, base, [[sw, 4], [s, h2], [1, s]])
                        v.tensor_tensor(
                            out,
                            ap_of(zb, base, [[sw, 4], [s, h2], [1, s]]),
                            ap_of(zb, base + hc * s, [[sw, 4], [s, h2], [1, s]]),
                            OP.add,
                        )
                    h = hc

            # ---- chart DP ----
            for w in range(1, N):
                s = N - w
                sw = s * w
                row = (w - 1) * N + 1
                # opA: NOCHILD edges (t=0 for q=R, t=w-1 for q=L) have a
                # trivial K-factor of 1, so products cover only the w-1
                # HASCHILD splits; the edges are read straight from banks.
                # tmp2[q,g,i] = NOCHILD edge * {A0,B0}[w,:]
                v.tensor_tensor(
                    ap_of(zb, ZB_T2, [[4 * s, 2], [s, 4], [1, s]]),
                    ap_of(banks, S_CLA * D + (w - 1) * N + 1,
                          [[-4 * D - 1, 2], [D, 4], [1, s]]),
                    ap_of(consts, C_A0 * D + w * N, [[D, 2], [4 * D, 4], [1, s]]),
                    OP.mult,
                )
                if w > 1:
                    sw1 = s * (w - 1)
                    # q=R forward, t in [1,w): P[g,t,i] = KR[t,i]*CLa[w-1-t, i+t+1]
                    v.tensor_tensor(
                        ap_of(zb, ZB_P, [[sw1, 4], [s, w - 1], [1, s]]),
                        ap_of(banks, S_KR * D + N, [[D, 4], [N, w - 1], [1, s]]),
                        ap_of(banks, S_CLA * D + row - 40, [[D, 4], [-40, w - 1], [1, s]]),
                        OP.mult,
                    )
                    # q=L t-reversed, t' in [1,w): P[4+g,t',i] = CRa[w-1-t',i]*KL[t', i+w-t']
                    v.tensor_tensor(
                        ap_of(zb, ZB_P + 4 * sw1, [[sw1, 4], [s, w - 1], [1, s]]),
                        ap_of(banks, S_CRA * D + (w - 2) * N, [[D, 4], [-N, w - 1], [1, s]]),
                        ap_of(banks, S_KL * D + N + w - 1, [[D, 4], [40, w - 1], [1, s]]),
                        OP.mult,
                    )
                    fold_pair(w - 1, s, 0, w - 1)
                    # tmp1[q,g,i] = (sum over HASCHILD splits) * {A1,B1}[w,:]
                    v.tensor_tensor(
                        ap_of(zb, ZB_T1, [[4 * s, 2], [s, 4], [1, s]]),
                        ap_of(zb, ZB_P, [[4 * sw1, 2], [sw1, 4], [1, s]]),
                        ap_of(consts, C_A1 * D + w * N, [[D, 2], [4 * D, 4], [1, s]]),
                        OP.mult,
                    )
                    # IR[w-1, i] = tmp1R + tmp2R; IL[w-1, i+1] = tmp1L + tmp2L
                    # (two ops so each opB mult waits only on its own input)
                    v.tensor_tensor(
                        ap_of(banks, S_IR * D + (w - 1) * N, [[D, 4], [1, s]]),
                        ap_of(zb, ZB_T1, [[s, 4], [1, s]]),
                        ap_of(zb, ZB_T2, [[s, 4], [1, s]]),
                        OP.add,
                    )
                    v.tensor_tensor(
                        ap_of(banks, S_IL * D + (w - 1) * N + 1, [[D, 4], [1, s]]),
                        ap_of(zb, ZB_T1 + 4 * s, [[s, 4], [1, s]]),
                        ap_of(zb, ZB_T2 + 4 * s, [[s, 4], [1, s]]),
                        OP.add,
                    )
                else:
                    # w=1: only the NOCHILD edge exists: IR/IL row 0 = tmp2
                    v.tensor_copy(
                        ap_of(banks, S_IR * D, [[4 * D + 1, 2], [D, 4], [1, s]]),
                        ap_of(zb, ZB_T2, [[4 * s, 2], [s, 4], [1, s]]),
                    )
                # opB products, half 0 (q=L): P[g,t,i] = CLa[t,i]*IL[w-1-t, i+t+1]
                # half 1 (q=R): P[4+g,t,i] = IR[t,i]*CRa[w-1-t, i+t+1]
                klout = ap_of(banks, S_KL * D + w * N, [[D, 4], [1, s]])
                krout = ap_of(banks, S_KR * D + w * N, [[D, 4], [1, s]])
                if w == 1:
                    v.tensor_tensor(
                        klout,
                        ap_of(banks, S_CLA * D, [[D, 4], [N, 1], [1, s]]),
                        ap_of(banks, S_IL * D + row, [[D, 4], [-40, 1], [1, s]]),
                        OP.mult,
                    )
                    v.tensor_tensor(
                        krout,
                        ap_of(banks, S_IR * D, [[D, 4], [N, 1], [1, s]]),
                        ap_of(banks, S_CRA * D + row, [[D, 4], [-40, 1], [1, s]]),
                        OP.mult,
                    )
                else:
                    v.tensor_tensor(
                        ap_of(zb, ZB_P, [[sw, 4], [s, w], [1, s]]),
                        ap_of(banks, S_CLA * D, [[D, 4], [N, w], [1, s]]),
                        ap_of(banks, S_IL * D + row, [[D, 4], [-40, w], [1, s]]),
                        OP.mult,
                    )
                    v.tensor_tensor(
                        ap_of(zb, ZB_P + 4 * sw, [[sw, 4], [s, w], [1, s]]),
                        ap_of(banks, S_IR * D, [[D, 4], [N, w], [1, s]]),
                        ap_of(banks, S_CRA * D + row, [[D, 4], [-40, w], [1, s]]),
                        OP.mult,
                    )
                    fold_pair(w, s, 0, w, final0=klout, final1=krout)
                # CRa[w,i] = KR[w,i]*sRhas[i]; CLa[w,i] = KL[w,i]*sLhas[i+w]
                v.tensor_tensor(
                    ap_of(banks, S_CRA * D + w * N, [[4 * D, 2], [D, 4], [1, s]]),
                    ap_of(banks, S_KR * D + w * N, [[-4 * D, 2], [D, 4], [1, s]]),
                    ap_of(stops, V_SRHAS * N, [[w - 2 * N, 2], [4 * N, 4], [1, s]]),
                    OP.mult,
                )

                if w == RENORM_W:
                    s0 = N - w
                    # mu[g] = max_i max(KR[w,i], KL[w,i])  (per partition)
                    v.tensor_reduce(
                        ap_of(zf, ZF_M2, [[4, 2], [1, 4]]),
                        ap_of(banks, S_KL * D + w * N, [[4 * D, 2], [D, 4], [1, s0]]),
                        axis=AX.X, op=OP.max,
                    )
                    v.tensor_tensor(
                        ap_of(zf, ZF_MU, [[1, 4]]),
                        ap_of(zf, ZF_M2, [[1, 4]]),
                        ap_of(zf, ZF_M2 + 4, [[1, 4]]),
                        OP.max,
                    )
                    # Ln range on ACT is +-2^64: compute via mu*2^-32
                    v.tensor_scalar_mul(
                        ap_of(zf, ZF_MU, [[1, 4]]), ap_of(zf, ZF_MU, [[1, 4]]), 2.0**-32
                    )
                    v.tensor_scalar_max(
                        ap_of(zf, ZF_MU, [[1, 4]]), ap_of(zf, ZF_MU, [[1, 4]]), 1e-36
                    )
                    sc.activation(
                        ap_of(zf, ZF_LM, [[1, 4]]), ap_of(zf, ZF_MU, [[1, 4]]), AF.Ln
                    )
                    # quantize the per-width shift to delta = -k*ln2 with k
                    # integer, so every rescale factor is an EXACT power of
                    # two (exact in bf16 as well).
                    # kf = round((log(mu*2^-32) + 32 ln2) / (w ln2))
                    v.tensor_scalar(
                        ap_of(zf, ZF_LM, [[1, 4]]), ap_of(zf, ZF_LM, [[1, 4]]),
                        LN2_32, 1.0 / (w * float(np.log(2.0))),
                        OP.add, OP.mult,
                    )
                    v.tensor_scalar(
                        ap_of(zf, ZF_LM, [[1, 4]]), ap_of(zf, ZF_LM, [[1, 4]]),
                        12582912.0, 12582912.0, OP.add, OP.subtract,
                    )
                    # dsum accumulates k (exact small integers)
                    v.tensor_tensor(
                        ap_of(zf, ZF_DSUM, [[1, 4]]),
                        ap_of(zf, ZF_DSUM, [[1, 4]]),
                        ap_of(zf, ZF_LM, [[1, 4]]),
                        OP.add,
                    )
                    # scale2 = 2^-k via exponent bits: (127 - k) << 23
                    v.tensor_scalar(
                        ap_of(zf, ZF_M2, [[1, 4]]), ap_of(zf, ZF_LM, [[1, 4]]),
                        -1.0, 127.0, OP.mult, OP.add,
                    )
                    zi = zf.bitcast(mybir.dt.int32)
                    v.tensor_copy(
                        ap_of(zi, ZF_M2 + 4, [[1, 4]]),
                        ap_of(zf, ZF_M2, [[1, 4]]),
                    )
                    v.tensor_scalar(
                        ap_of(zi, ZF_M2 + 4, [[1, 4]]),
                        ap_of(zi, ZF_M2 + 4, [[1, 4]]),
                        23, None, OP.arith_shift_left,
                    )
                    # M[g, d] = 2^(-k*d): d=0 -> 1, then multiplicative scan
                    v.memset(ap_of(zf, ZF_M, [[42, 4], [1, 1]]), 1.0)
                    for g in range(G):
                        sca = ap_of(zf, ZF_M2 + 4 + g, [[0, 41]])
                        v.tensor_tensor_scan(
                            ap_of(zf, ZF_M + g * 42 + 1, [[1, 41]]),
                            sca, sca, 1.0, OP.mult, OP.bypass,
                        )
                    # expand to Mx[g, d, i] = M[g, d] (packed bf16, exact
                    # powers of two) so the rescales hit the 2x DVE mode
                    v.tensor_copy(
                        ap_of(zb, ZB_MX, [[(w + 2) * N, 4], [N, w + 2], [1, N]]),
                        ap_of(zf, ZF_M, [[42, 4], [1, w + 2], [0, N]]),
                    )
                    # far const rows (needed from w=26 on) rescale on Pool,
                    # overlapping the DVE rescales and the next few widths
                    for g in range(G):
                        tCf = ap_of(consts, 4 * g * D + 26 * N,
                                    [[D, 4], [N, N - 26], [1, N]])
                        nc.gpsimd.tensor_tensor(
                            tCf, tCf,
                            ap_of(zb, ZB_MX + g * (w + 2) * N + N,
                                  [[0, 4], [0, N - 26], [1, N]]),
                            OP.mult,
                        )
                    for g in range(G):
                        mg = ZB_MX + g * (w + 2) * N
                        # natural tables, rows d<=w: scale by 2^(-k*d)
                        tA = ap_of(banks, g * D, [[4 * D, 4], [N, w + 1], [1, N]])
                        v.tensor_tensor(
                            tA, tA,
                            ap_of(zb, mg, [[0, 4], [N, w + 1], [1, N]]),
                            OP.mult,
                        )
                        # IR/IL rows r<=w-1 hold width r+1: scale 2^(-k*(r+1))
                        tI = ap_of(banks, (16 + g) * D, [[4 * D, 2], [N, w], [1, N]])
                        v.tensor_tensor(
                            tI, tI,
                            ap_of(zb, mg + N, [[0, 2], [N, w], [1, N]]),
                            OP.mult,
                        )
                        # near const rows w+1..25: one extra arc factor 2^-k
                        tC = ap_of(consts, 4 * g * D + (w + 1) * N,
                                   [[D, 4], [N, 25 - w], [1, N]])
                        v.tensor_tensor(
                            tC, tC,
                            ap_of(zb, mg + N, [[0, 4], [0, 25 - w], [1, N]]),
                            OP.mult,
                        )

            # ---- extract raw exp-domain CR[0, j] (log on host) ----
            v.tensor_copy(
                ap_of(zf, ZF_CROUT, [[N, 4], [1, N]]),
                ap_of(banks, S_CRA * D, [[D, 4], [N, N]]),
            )
            nc.sync.dma_start(
                ap_of(logs_d, 0, [[N, G], [1, N]], lead=[G * N, 128]),
                ap_of(zf, ZF_CROUT, [[N, G], [1, N]]),
            )
            nc.sync.dma_start(
                ap_of(dsum_d, 0, [[1, G]], lead=[G, 128]),
                ap_of(zf, ZF_DSUM, [[1, G]]),
            )

    nc.compile()
    return nc


_NC_CACHE = {}


def get_nc():
    if "nc" not in _NC_CACHE:
        _NC_CACHE["nc"] = build_nc()
    return _NC_CACHE["nc"]


def make_in_maps(trans_scores, dec_scores):
    t = np.asarray(trans_scores, dtype=np.float32)
    dec = np.asarray(dec_scores, dtype=np.float32)
    B = t.shape[0]
    go = dec[..., 0]                        # [B, n, dir, dv]
    # per-sentence linear pre-shift: each arc factor carries exp(-c0), so a
    # width-w entry is scaled exp(-c0*w); undone on the host at the end.
    tm = np.where(t < -1e8, -np.inf, t).max(axis=3)
    with np.errstate(invalid="ignore"):
        colmax = tm.max(axis=1)             # [B, n] best arc into each child
        proxy = np.nanmean(
            np.where(np.isfinite(colmax), colmax, np.nan)[:, 1:], axis=-1)
    c0 = (proxy + 0.5).astype(np.float32)
    c0 = np.clip(np.nan_to_num(c0), -20.0, 20.0)
    # one exp over trans (NEG -> 0 underflow is intended), then gather diags
    with np.errstate(under="ignore"):
        E = np.exp(t - c0[:, None, None, None])      # [B, n, n, 2]
        ego = np.exp(go)                             # [B, n, 2, 2]
    d_idx, i_idx = np.meshgrid(np.arange(N), np.arange(N), indexing="ij")
    j_idx = np.minimum(i_idx + d_idx, N - 1)
    valid = ((i_idx + d_idx) <= N - 1)[None].astype(np.float32)
    ea = E[:, i_idx, j_idx, :]              # [B, n, n, 2]  trans[i, i+d, v]
    eb = E[:, j_idx, i_idx, :]              # [B, n, n, 2]  trans[i+d, i, v]
    a1 = ea[..., 1] * ego[:, :, 1, 1][:, i_idx] * valid
    a0 = ea[..., 0] * ego[:, :, 1, 0][:, i_idx] * valid
    b1 = eb[..., 1] * ego[:, :, 0, 1][:, j_idx] * valid
    b0 = eb[..., 0] * ego[:, :, 0, 0][:, j_idx] * valid
    consts = np.empty((B, 4, N, N), dtype=np.float32)
    consts[:, 0] = a1
    consts[:, 1] = b1
    consts[:, 2] = a0
    consts[:, 3] = b0
    consts = consts.reshape(B, CONST_IN).astype(ml_dtypes.bfloat16)
    est = np.exp(dec[..., 1])               # [B, n, dir, dv]
    stops = np.empty((B, 4, N), dtype=np.float32)
    stops[:, 0] = est[:, :, 0, 0]; stops[:, 1] = est[:, :, 0, 1]
    stops[:, 2] = est[:, :, 1, 0]; stops[:, 3] = est[:, :, 1, 1]
    stops = stops.reshape(B, STOP_IN).astype(ml_dtypes.bfloat16)
    in_maps = []
    for c in range(NCORES):
        sl = slice(c * B_CORE, (c + 1) * B_CORE)
        in_maps.append({
            "consts": consts[sl],
            "stops": stops[sl],
        })
    return in_maps, c0


L0_HOST = 5  # sentences with len <= L0_HOST are computed exactly on the host


def _host_short_ll(trans, dec, lens):
    """Exact f64 LL for short sentences via the inside DP truncated to
    positions 0..L0_HOST (spans of a length-l sentence live within [0, l])."""
    n = L0_HOST + 1
    t = np.asarray(trans)[:, :n, :n, :].astype(np.float64)
    dc = np.asarray(dec)[:, :n].astype(np.float64)
    B = t.shape[0]
    go = dc[..., 0]
    stop = dc[..., 1]
    NEG = -1e9
    IR = np.full((B, n, n), NEG)
    IL = np.full((B, n, n), NEG)
    KR = np.full((B, n, n), NEG)
    KL = np.full((B, n, n), NEG)
    dg = np.arange(n)
    KR[:, dg, dg] = 0.0
    KL[:, dg, dg] = 0.0
    CR = np.full((B, n, n), NEG)
    CL = np.full((B, n, n), NEG)
    CR[:, dg, dg] = stop[:, :, 1, 0]
    CL[:, dg, dg] = stop[:, :, 0, 0]
    goR = go[:, :, 1, :]
    goL = go[:, :, 0, :]

    def lse(x):
        m = x.max(axis=-1, keepdims=True)
        return np.squeeze(m, -1) + np.log(np.exp(x - m).sum(axis=-1))

    for w in range(1, n):
        s = n - w
        i = np.arange(s)[:, None]
        tt = np.arange(w)[None, :]
        j = i + w
        k = i + tt
        vR = (tt > 0).astype(np.int64)
        ir = lse(KR[:, i, k] + goR[:, i, vR] + t[:, i, j, np.minimum(vR, 1)]
                 + CL[:, k + 1, j])
        vL = (tt < w - 1).astype(np.int64)
        il = lse(CR[:, i, k] + KL[:, k + 1, j] + goL[:, j, vL]
                 + t[:, j, i, np.minimum(vL, 1)])
        i1 = np.arange(s)
        IR[:, i1, i1 + w] = ir
        IL[:, i1, i1 + w] = il
        kr = lse(IR[:, i, i + 1 + tt] + CR[:, i + 1 + tt, j])
        kl = lse(CL[:, i, i + tt] + IL[:, i + tt, j])
        KR[:, i1, i1 + w] = kr
        KL[:, i1, i1 + w] = kl
        CR[:, i1, i1 + w] = kr + stop[:, i1, 1, 1]
        CL[:, i1, i1 + w] = kl + stop[:, i1 + w, 0, 1]

    return CR[np.arange(B), 0, lens].astype(np.float32)


def assemble(results, len_array, c0):
    ln = np.asarray(len_array).astype(np.int64)
    c0 = np.asarray(c0).astype(np.float64)
    out = np.empty(len(ln), dtype=np.float32)
    for c, res in enumerate(results):
        ecr = res["ecr"].reshape(B_CORE, N).astype(np.float64)
        dsum = res["dsum"].reshape(B_CORE).astype(np.float64)
        lc = ln[c * B_CORE:(c + 1) * B_CORE]
        idx = np.arange(B_CORE)
        with np.errstate(divide="ignore"):
            out[c * B_CORE:(c + 1) * B_CORE] = (
                np.log(ecr[idx, lc]) + dsum * np.log(2.0) * lc
                + c0[c * B_CORE:(c + 1) * B_CORE] * lc
            ).astype(np.float32)
    return out


def kernel(trans_scores, dec_scores, len_array):
    from concourse.bass_utils import run_bass_kernel_spmd

    nc = get_nc()
    in_maps, c0 = make_in_maps(trans_scores, dec_scores)
    res = run_bass_kernel_spmd(nc, in_maps, core_ids=list(range(NCORES)))
    out = assemble(res.results, len_array, c0)
    lens = np.asarray(len_array).astype(np.int64)
    short = lens <= L0_HOST
    if short.any():
        out[short] = _host_short_ll(
            np.asarray(trans_scores)[short], np.asarray(dec_scores)[short],
            lens[short])
    return out
